# revision 8
# baseline (speedup 1.0000x reference)
"""Trainium2 Bass kernel for the descriptor-module MoE-routing problem.

Computes, for coords (2048, 64, 3), atom_types (2048, 64), W (3, 384):
    desc_sorted    = tanh(coords.reshape(-1, 3) @ W)[argsort_stable(types)]
    at2conf_sorted = at2conf[argsort_stable(types)]
    at_type_count  = bincount(types, 4)

Sharding: data-parallel over conformers — 8 cores x 256 conformers
(16384 atoms each). The stable sort by type decomposes: the global
bucket for type t is the concatenation (in shard order) of the per-shard
buckets for type t, so each core routes its shard independently and the
host stitches buckets back together.

Per-core device algorithm (N=16384 atoms = 128 tiles x 128):
  1. dst[i] (sorted position) via counting sort computed with matmuls:
     per-type masks, within-tile exclusive ranks (strict-triangular
     matmul), tile prefix sums (second triangular matmul), bucket
     offsets. All in f32 (values < 16384, exact).
  2. One-pass routing with the dma_scatter_add Q7 custom instruction
     (the generic vector-indirect DGE path is disabled in this
     toolchain): 16-byte records [x, y, z, conf] are scatter-added at
     256-byte row stride into a zeroed table[dst[i]]. dst is a
     permutation, so each row is written exactly once.
  3. Streaming epilogue: read routed records back, PE-transpose each
     128-atom tile, K=3 matmul with W, tanh on ScalarE, contiguous
     write of desc_sorted (the 25 MB/core memory-roofline stream).
"""

import sys

import numpy as np

try:
    import concourse.bass as bass  # noqa: F401
except ImportError:  # pragma: no cover
    for p in ("/opt/trn_rl_repo", "/root/.axon_site/_ro/trn_rl_repo"):
        if p not in sys.path:
            sys.path.insert(0, p)
    import concourse.bass as bass  # noqa: F401

import concourse.bacc as bacc
import concourse.tile as tile
from concourse import mybir
from concourse.bass_utils import run_bass_kernel_spmd
from concourse.mybir import ActivationFunctionType, AluOpType

N_CORES = 8
N_CONFS = 2048
N_ATOMS = 64
NTYPES = 4
NDESC = 384
N = N_CONFS * N_ATOMS  # 131072
NS = N // N_CORES  # 16384 atoms per core
P = 128
KT = NS // P  # 128 tiles per core
REC = 64  # table row stride in f32 (256 B, dma_scatter_add minimum)
F32 = mybir.dt.float32
I16 = mybir.dt.int16
I32 = mybir.dt.int32

_CACHE = {}


def build_program():
    """Build + compile the single-core Bass program (run SPMD on 8 cores)."""
    if "nc" in _CACHE:
        return _CACHE["nc"]

    nc = bacc.Bacc("TRN2", target_bir_lowering=False, debug=False)

    coords_in = nc.dram_tensor("coords", [NS, 3], F32, kind="ExternalInput")
    types_in = nc.dram_tensor("types", [KT, P], I32, kind="ExternalInput")
    conf_in = nc.dram_tensor("confg", [KT, P], F32, kind="ExternalInput")
    w_in = nc.dram_tensor("w_in", [3, NDESC], F32, kind="ExternalInput")
    lmat_in = nc.dram_tensor("lmat_in", [P, P], F32, kind="ExternalInput")
    ident_in = nc.dram_tensor("ident_in", [P, P], F32, kind="ExternalInput")

    table = nc.dram_tensor("table", [NS, REC], F32)  # routed records scratch

    desc_out = nc.dram_tensor("desc_out", [NS, NDESC], F32, kind="ExternalOutput")
    conf_out = nc.dram_tensor("conf_out", [P, KT], F32, kind="ExternalOutput")
    cnt_out = nc.dram_tensor("cnt_out", [1, NTYPES], F32, kind="ExternalOutput")

    with tile.TileContext(nc) as tc, \
            tc.tile_pool(name="const", bufs=1) as cpool, \
            tc.tile_pool(name="setup", bufs=1) as spool:
        setup_psum = tc.tile_pool(name="spsum", bufs=2, space="PSUM")
        sppool = setup_psum.__enter__()
        rank_psum = tc.tile_pool(name="rankps", bufs=1, space="PSUM")
        rkpool = rank_psum.__enter__()

        # ---- constants ----
        w_s = cpool.tile([P, NDESC], F32, tag="w")
        nc.sync.dma_start(out=w_s[0:3, :], in_=w_in.ap())
        lmat = cpool.tile([P, P], F32, tag="lmat")
        nc.sync.dma_start(out=lmat[:], in_=lmat_in.ap())
        ident = cpool.tile([P, P], F32, tag="ident")
        nc.sync.dma_start(out=ident[:], in_=ident_in.ap())
        ones = cpool.tile([P, P], F32, tag="ones")
        nc.vector.memset(ones[:], 1.0)

        # ---- zero the routing table (cols 0:4 must be 0 for the adds) ----
        zeros = spool.tile([P, 2048], F32, tag="zeros")
        nc.vector.memset(zeros[:], 0.0)
        table_z = table.ap().rearrange("(p f) c -> p (f c)", p=P)
        for q in range(4):
            nc.sync.dma_start(
                out=table_z[:, q * 2048:(q + 1) * 2048], in_=zeros[:]
            )

        # ---- inputs: [k, a] layout (tile k = partition, atom a = free) ----
        t_i = spool.tile([P, P], I32, tag="t_i")
        nc.sync.dma_start(out=t_i[:], in_=types_in.ap())
        t_f = spool.tile([P, P], F32, tag="t_f")
        nc.vector.tensor_copy(out=t_f[:], in_=t_i[:])
        c_ka = spool.tile([P, P, 3], F32, tag="c_ka")
        nc.sync.dma_start(
            out=c_ka[:], in_=coords_in.ap().rearrange("(k a) c -> k a c", k=P)
        )
        f_ka = spool.tile([P, P], F32, tag="f_ka")
        nc.sync.dma_start(out=f_ka[:], in_=conf_in.ap())

        # ---- types in [a, k] layout for rank matmuls ----
        t_ak_p = sppool.tile([P, P], F32, tag="ps")
        nc.tensor.transpose(out=t_ak_p[:], in_=t_f[:], identity=ident[:])
        t_ak = spool.tile([P, P], F32, tag="t_ak")
        nc.vector.tensor_copy(out=t_ak[:], in_=t_ak_p[:])

        # ---- per-type masks (stacked on free dim), ranks ----
        # lmat[r, c] = 1 iff r < c, so (lmat^T . m)[a, k] = sum_{a'<a} m[a', k]
        m4 = spool.tile([P, NTYPES, P], F32, tag="m4")
        rank_ps = []
        for t in range(NTYPES):
            nc.vector.tensor_scalar(
                out=m4[:, t, :], in0=t_ak[:], scalar1=float(t), scalar2=None,
                op0=AluOpType.is_equal,
            )
            # exclusive within-tile rank; group stays open for the O add below
            rp = rkpool.tile([P, P], F32, tag=f"rank{t}")
            nc.tensor.matmul(
                out=rp[:], lhsT=lmat[:], rhs=m4[:, t, :], start=True, stop=False
            )
            rank_ps.append(rp)

        # ---- per-(type, tile) counts: one N=512 matmul, then column
        # transposes to land cntT[k, t] with everything at partition 0 ----
        cntf_p = sppool.tile([1, NTYPES * P], F32, tag="ps")
        nc.tensor.matmul(
            out=cntf_p[:], lhsT=ones[:, 0:1], rhs=m4[:], start=True, stop=True
        )
        cntf = spool.tile([1, NTYPES * P], F32, tag="cntf")
        nc.vector.tensor_copy(out=cntf[:], in_=cntf_p[:])
        cntT = spool.tile([P, NTYPES], F32, tag="cntT")
        for t in range(NTYPES):
            col_p = sppool.tile([P, 1], F32, tag="ps")
            nc.tensor.transpose(
                out=col_p[:], in_=cntf[0:1, t * P:(t + 1) * P],
                identity=ident[0:1, 0:1],
            )
            nc.vector.tensor_copy(out=cntT[:, t:t + 1], in_=col_p[:])
        tot_p = sppool.tile([1, NTYPES], F32, tag="ps")
        nc.tensor.matmul(
            out=tot_p[:], lhsT=ones[:, 0:1], rhs=cntT[:], start=True, stop=True
        )
        tot = spool.tile([1, NTYPES], F32, tag="tot")
        nc.vector.tensor_copy(out=tot[:], in_=tot_p[:])
        nc.sync.dma_start(out=cnt_out.ap(), in_=tot[:])
        bofs = spool.tile([P, NTYPES], F32, tag="bofs")
        nc.vector.memset(bofs[0:1, :], 0.0)
        for t in range(1, NTYPES):
            nc.vector.tensor_add(
                out=bofs[0:1, t:t + 1], in0=bofs[0:1, t - 1:t], in1=tot[0:1, t - 1:t]
            )
        # O[k, t] = bucket_offset[t] + tiles-before-k count, via PSUM accum
        o_kt_p = sppool.tile([P, NTYPES], F32, tag="ps")
        nc.tensor.matmul(
            out=o_kt_p[:], lhsT=lmat[:], rhs=cntT[:], start=True, stop=False
        )
        nc.tensor.matmul(
            out=o_kt_p[:], lhsT=ones[0:1, :], rhs=bofs[0:1, :],
            start=False, stop=True,
        )
        o_kt = spool.tile([P, NTYPES], F32, tag="o_kt")
        nc.vector.tensor_copy(out=o_kt[:], in_=o_kt_p[:])
        # transpose each O column so every O row sits at partition 0
        o_rows = []
        for t in range(NTYPES):
            orow_p = sppool.tile([1, P], F32, tag="ps")
            nc.tensor.transpose(
                out=orow_p[:], in_=o_kt[:, t:t + 1], identity=ident[:]
            )
            o_row = spool.tile([1, P], F32, tag=f"orow{t}")
            nc.vector.tensor_copy(out=o_row[:], in_=orow_p[:])
            o_rows.append(o_row)

        # ---- dst[a, k] = sum_t mask_t * (rank_t + O[t, k]) ----
        # close each rank group by accumulating ones (x) O_row_t, then mask
        dst_ak = spool.tile([P, P], F32, tag="dst_ak")
        for t in range(NTYPES):
            nc.tensor.matmul(
                out=rank_ps[t][:], lhsT=ones[0:1, :], rhs=o_rows[t][0:1, :],
                start=False, stop=True,
            )
            tmp = spool.tile([P, P], F32, tag="tmp")
            nc.vector.tensor_tensor(
                out=tmp[:], in0=rank_ps[t][:], in1=m4[:, t, :], op=AluOpType.mult
            )
            if t == 0:
                nc.vector.tensor_copy(out=dst_ak[:], in_=tmp[:])
            else:
                nc.vector.tensor_add(out=dst_ak[:], in0=dst_ak[:], in1=tmp[:])
        dst_ka_p = sppool.tile([P, P], F32, tag="ps")
        nc.tensor.transpose(out=dst_ka_p[:], in_=dst_ak[:], identity=ident[:])
        dst_ka = spool.tile([P, P], F32, tag="dst_ka")
        nc.vector.tensor_copy(out=dst_ka[:], in_=dst_ka_p[:])

        # ---- int16 index tile for dma_scatter_add ----
        # Processing order j = a*128 + k; the ucode wraps indices 16-wide:
        # idxs[p, j//16] = dst[k, a] with k = (j//16 % 8)*16 + p%16 and
        # a = j//128, i.e. idx16[p, a, q] = dst_ka[q*16 + p%16, a]. Rows
        # 0:16 come from identity-slice matmul extractions; rows 16:128
        # are DMA-replicated (DVE cannot shift partitions).
        idx16f = spool.tile([P, P, 8], F32, tag="idx16f")
        for q in range(8):
            ext_p = sppool.tile([16, P], F32, tag="ps")
            nc.tensor.matmul(
                out=ext_p[:], lhsT=ident[:, q * 16:(q + 1) * 16], rhs=dst_ka[:],
                start=True, stop=True,
            )
            nc.vector.tensor_copy(out=idx16f[0:16, :, q], in_=ext_p[:])
        idx16 = spool.tile([P, P, 8], I16, tag="idx16")
        nc.vector.tensor_copy(out=idx16[0:16, :, :], in_=idx16f[0:16, :, :])
        for lo, hi in ((16, 32), (32, 64), (64, 128)):
            nc.sync.dma_start(
                out=idx16[lo:hi, :, :], in_=idx16[lo - (hi - lo):lo, :, :]
            )

        # ---- records [x, y, z, conf] and the routing scatter ----
        rec = spool.tile([P, P, 4], F32, tag="rec")
        nc.vector.tensor_copy(out=rec[:, :, 0:3], in_=c_ka[:])
        nc.vector.tensor_copy(
            out=rec[:, :, 3:4], in_=f_ka[:].rearrange("p (a o) -> p a o", o=1)
        )
        AC = 32  # atom columns per scatter chunk -> 4096 indices
        for ac in range(0, P, AC):
            nc.gpsimd.dma_scatter_add(
                table.ap()[:, 0:4],
                rec[:, ac:ac + AC, :],
                idx16[:, ac:ac + AC, :],
                P * AC,
                P * AC,
                4,
                elem_step=REC,
            )

        rank_psum.__exit__(None, None, None)
        setup_psum.__exit__(None, None, None)

        # ---- streaming epilogue: table -> desc_sorted ----
        table_r = table.ap().rearrange("(t p) c -> p t c", p=P)
        desc_r = desc_out.ap().rearrange("(t p) d -> p t d", p=P)
        conf_pk = spool.tile([P, KT], F32, tag="conf_pk")
        with tc.tile_pool(name="rb", bufs=3) as rbpool, \
                tc.tile_pool(name="trs", bufs=3) as trspool, \
                tc.tile_pool(name="descp", bufs=3) as descpool, \
                tc.tile_pool(name="mm", bufs=3, space="PSUM") as mmpool, \
                tc.tile_pool(name="trp", bufs=3, space="PSUM") as trppool:
            for g16 in range(KT // 16):
                rb = rbpool.tile([P, 16, 4], F32, tag="rb")
                nc.sync.dma_start(
                    out=rb[:], in_=table_r[:, g16 * 16:(g16 + 1) * 16, 0:4]
                )
                nc.vector.tensor_copy(
                    out=conf_pk[:, g16 * 16:(g16 + 1) * 16], in_=rb[:, :, 3]
                )
                for g4 in range(4):
                    dt_ = descpool.tile([P, 4, NDESC], F32, tag="desc")
                    for j in range(4):
                        jj = g4 * 4 + j
                        trp = trppool.tile([4, P], F32, tag="trp")
                        nc.tensor.transpose(
                            out=trp[:], in_=rb[:, jj, :], identity=ident[:]
                        )
                        trs = trspool.tile([P, P], F32, tag="trs")
                        nc.vector.tensor_copy(out=trs[0:4, :], in_=trp[:])
                        mm = mmpool.tile([P, NDESC], F32, tag="mm")
                        nc.tensor.matmul(
                            out=mm[:], lhsT=trs[0:3, :], rhs=w_s[0:3, :],
                            start=True, stop=True,
                        )
                        nc.scalar.activation(
                            out=dt_[:, j, :], in_=mm[:],
                            func=ActivationFunctionType.Tanh,
                        )
                    g = g16 * 4 + g4
                    nc.sync.dma_start(
                        out=desc_r[:, g * 4:(g + 1) * 4, :], in_=dt_[:]
                    )
        nc.sync.dma_start(out=conf_out.ap(), in_=conf_pk[:])

    nc.compile()
    _CACHE["nc"] = nc
    return nc


def make_in_maps(coords, atom_types, W):
    """Shard full inputs into the 8 per-core input maps."""
    coords_flat = np.ascontiguousarray(
        np.asarray(coords, dtype=np.float32).reshape(N, 3)
    )
    types_flat = np.asarray(atom_types).reshape(N).astype(np.int32)
    w_np = np.ascontiguousarray(np.asarray(W, dtype=np.float32))
    lmat = np.triu(np.ones((P, P), dtype=np.float32), k=1)
    ident = np.eye(P, dtype=np.float32)
    conf_all = (np.arange(N, dtype=np.int64) // N_ATOMS).astype(np.float32)

    in_maps = []
    for s in range(N_CORES):
        sl = slice(s * NS, (s + 1) * NS)
        in_maps.append({
            "coords": coords_flat[sl],
            "types": np.ascontiguousarray(types_flat[sl].reshape(KT, P)),
            "confg": np.ascontiguousarray(conf_all[sl].reshape(KT, P)),
            "w_in": w_np,
            "lmat_in": lmat,
            "ident_in": ident,
        })
    return in_maps


def assemble(results):
    """Stitch 8 per-core outputs into the full (desc, conf, counts) tuple."""
    counts = np.stack([
        np.rint(res["cnt_out"]).astype(np.int64).reshape(NTYPES)
        for res in results
    ])  # (8, 4)
    at_type_count = counts.sum(axis=0).astype(np.int32)

    conf_sorted = [
        np.rint(res["conf_out"].T.reshape(NS)).astype(np.int32)
        for res in results
    ]
    # local bucket start offsets per core (exclusive cumsum over types)
    loc_off = np.concatenate(
        [np.zeros((N_CORES, 1), np.int64), np.cumsum(counts, axis=1)], axis=1
    )
    desc_full = np.empty((N, NDESC), dtype=np.float32)
    conf_full = np.empty(N, dtype=np.int32)
    pos = 0
    for t in range(NTYPES):
        for s in range(N_CORES):
            c = int(counts[s, t])
            lo = int(loc_off[s, t])
            desc_full[pos:pos + c] = results[s]["desc_out"][lo:lo + c]
            conf_full[pos:pos + c] = conf_sorted[s][lo:lo + c]
            pos += c
    assert pos == N
    return desc_full, conf_full, at_type_count


def kernel(coords, atom_types, W):
    nc = build_program()
    in_maps = make_in_maps(coords, atom_types, W)
    res = run_bass_kernel_spmd(nc, in_maps, list(range(N_CORES)))
    return assemble(res.results)


# revision 13
# speedup vs baseline: 1.2829x; 1.2829x over previous
"""Trainium2 Bass kernel for the descriptor-module MoE-routing problem.

Computes, for coords (2048, 64, 3), atom_types (2048, 64), W (3, 384):
    desc_sorted    = tanh(coords.reshape(-1, 3) @ W)[argsort_stable(types)]
    at2conf_sorted = at2conf[argsort_stable(types)]
    at_type_count  = bincount(types, 4)

Sharding: data-parallel over conformers — 8 cores x 256 conformers
(16384 atoms each), and each core further splits its shard into SS=8
independent sub-shards of 16 tiles (2048 atoms). A stable sort by type
decomposes over contiguous index ranges: the global bucket for type t is
the concatenation (in range order) of the per-range buckets, so every
(core, sub-shard) routes independently and the host stitches the 64
bucket lists back together. Sub-sharding lets the Q7 scatter stream of
sub-shard s+1 overlap the compute/write epilogue of sub-shard s.

Per-sub-shard device algorithm (2048 atoms = 16 tiles x 128):
  1. dst[i] (sorted position) via counting sort computed with matmuls:
     per-type masks, within-tile exclusive ranks (strict-triangular
     matmul), tile prefix sums (second triangular matmul), bucket
     offsets. All in f32 (values < 2048, exact).
  2. One-pass routing with the dma_scatter_add Q7 custom instruction
     (the generic vector-indirect DGE path is disabled in this
     toolchain): 16-byte records [x, y, z, conf] are scatter-added at
     256-byte row stride into a zeroed slice of table[dst[i]]. dst is a
     permutation, so each row is written exactly once.
  3. Streaming epilogue: read routed records back, PE-transpose each
     128-atom tile, K=3 float32r matmul with W, tanh on ScalarE,
     contiguous write of desc_sorted (25 MB/core roofline stream).
"""

import sys

import numpy as np

try:
    import concourse.bass as bass  # noqa: F401
except ImportError:  # pragma: no cover
    for p in ("/opt/trn_rl_repo", "/root/.axon_site/_ro/trn_rl_repo"):
        if p not in sys.path:
            sys.path.insert(0, p)
    import concourse.bass as bass  # noqa: F401

import concourse.bacc as bacc
import concourse.tile as tile
from concourse import mybir
from concourse.bass_utils import run_bass_kernel_spmd
from concourse.mybir import ActivationFunctionType, AluOpType

N_CORES = 8
N_CONFS = 2048
N_ATOMS = 64
NTYPES = 4
NDESC = 384
N = N_CONFS * N_ATOMS  # 131072
NS = N // N_CORES  # 16384 atoms per core
P = 128
KT = NS // P  # 128 tiles per core
SS = 8  # sub-shards per core (independent sorts)
KS = KT // SS  # 16 tiles per sub-shard
NSS = KS * P  # 2048 atoms per sub-shard
REC = 64  # table row stride in f32 (256 B, dma_scatter_add minimum)
F32 = mybir.dt.float32
F32R = mybir.dt.float32r
I16 = mybir.dt.int16
I32 = mybir.dt.int32

_CACHE = {}


def build_program():
    """Build + compile the single-core Bass program (run SPMD on 8 cores)."""
    if "nc" in _CACHE:
        return _CACHE["nc"]

    nc = bacc.Bacc("TRN2", target_bir_lowering=False, debug=False)

    coords_in = nc.dram_tensor("coords", [NS, 3], F32, kind="ExternalInput")
    types_in = nc.dram_tensor("types", [KT, P], I32, kind="ExternalInput")
    conf_in = nc.dram_tensor("confg", [KT, P], F32, kind="ExternalInput")
    w_in = nc.dram_tensor("w_in", [3, NDESC], F32, kind="ExternalInput")
    lmat_in = nc.dram_tensor("lmat_in", [P, P], F32, kind="ExternalInput")
    ident_in = nc.dram_tensor("ident_in", [P, P], F32, kind="ExternalInput")

    table = nc.dram_tensor("table", [NS, REC], F32)  # routed records scratch

    desc_out = nc.dram_tensor("desc_out", [NS, NDESC], F32, kind="ExternalOutput")
    conf_out = nc.dram_tensor("conf_out", [P, KT], F32, kind="ExternalOutput")
    cnt_out = nc.dram_tensor("cnt_out", [SS, NTYPES], F32, kind="ExternalOutput")

    table_z = table.ap().rearrange("(s p f) c -> s p (f c)", s=SS, p=P)
    table_r = table.ap().rearrange("(t p) c -> p t c", p=P)
    desc_r = desc_out.ap().rearrange("(t p) d -> p t d", p=P)
    coords_r = coords_in.ap().rearrange("(k a) c -> k a c", k=KT)

    with tile.TileContext(nc) as tc, \
            tc.tile_pool(name="const", bufs=1) as cpool, \
            tc.tile_pool(name="persist", bufs=1) as ppool, \
            tc.tile_pool(name="setup", bufs=2) as spool, \
            tc.tile_pool(name="spsum", bufs=2, space="PSUM") as sppool, \
            tc.tile_pool(name="rankps", bufs=2, space="PSUM") as rkpool, \
            tc.tile_pool(name="rb", bufs=3) as rbpool, \
            tc.tile_pool(name="trs", bufs=4) as trspool, \
            tc.tile_pool(name="descp", bufs=3) as descpool, \
            tc.tile_pool(name="mm", bufs=2, space="PSUM") as mmpool, \
            tc.tile_pool(name="trp", bufs=2, space="PSUM") as trppool:

        # ---- constants ----
        w_s = cpool.tile([P, NDESC], F32, tag="w")
        nc.sync.dma_start(out=w_s[0:3, :], in_=w_in.ap())
        w_r = cpool.tile([P, NDESC], F32R, tag="wr")
        nc.vector.tensor_copy(out=w_r[0:3, :], in_=w_s[0:3, :])
        lmat = cpool.tile([P, P], F32, tag="lmat")
        nc.sync.dma_start(out=lmat[:], in_=lmat_in.ap())
        ident = cpool.tile([P, P], F32, tag="ident")
        nc.sync.dma_start(out=ident[:], in_=ident_in.ap())
        ones = cpool.tile([P, P], F32, tag="ones")
        nc.vector.memset(ones[:], 1.0)
        zeros = cpool.tile([P, REC * NSS // P], F32, tag="zeros")
        nc.vector.memset(zeros[:], 0.0)

        # whole-shard loads, [k, a] layout (tile k = partition, atom a = free)
        t_i = ppool.tile([P, P], I32, tag="t_i")
        nc.sync.dma_start(out=t_i[:], in_=types_in.ap())
        t_f = ppool.tile([P, P], F32, tag="t_f")
        nc.vector.tensor_copy(out=t_f[:], in_=t_i[:])
        c_ka = ppool.tile([P, P, 3], F32, tag="c_ka")
        nc.sync.dma_start(out=c_ka[:], in_=coords_r)
        f_ka = ppool.tile([P, P], F32, tag="f_ka")
        nc.sync.dma_start(out=f_ka[:], in_=conf_in.ap())

        # transpose to [a, k] layout: types, coords planes, conf
        t_ak = ppool.tile([P, P], F32, tag="t_ak")
        tp = sppool.tile([P, P], F32, tag="ps")
        nc.tensor.transpose(out=tp[:], in_=t_f[:], identity=ident[:])
        nc.vector.tensor_copy(out=t_ak[:], in_=tp[:])
        # records in [a, (k, c)] layout so sub-shards slice on the free dim
        rec = ppool.tile([P, P, 4], F32, tag="rec")
        for c in range(3):
            cp_ = sppool.tile([P, P], F32, tag="ps")
            nc.tensor.transpose(
                out=cp_[:], in_=c_ka[:, :, c], identity=ident[:]
            )
            nc.vector.tensor_copy(out=rec[:, :, c], in_=cp_[:])
        fp_ = sppool.tile([P, P], F32, tag="ps")
        nc.tensor.transpose(out=fp_[:], in_=f_ka[:], identity=ident[:])
        nc.vector.tensor_copy(out=rec[:, :, 3], in_=fp_[:])

        conf_pk = ppool.tile([P, KT], F32, tag="conf_pk")

        for s in range(SS):
            ksl = slice(s * KS, (s + 1) * KS)  # tile (k) range of sub-shard

            # zero this sub-shard's table slice
            nc.sync.dma_start(out=table_z[s], in_=zeros[:])

            # ---- per-type masks (free-dim stacked) ----
            m4 = spool.tile([P, NTYPES, KS], F32, tag="m4")
            for t in range(NTYPES):
                nc.vector.tensor_scalar(
                    out=m4[:, t, :], in0=t_ak[:, ksl], scalar1=float(t),
                    scalar2=None, op0=AluOpType.is_equal,
                )

            # ---- counts cnt[t, k] -> cntT[k, t] at partition 0 ----
            cntf_p = sppool.tile([1, NTYPES * KS], F32, tag="ps")
            nc.tensor.matmul(
                out=cntf_p[:], lhsT=ones[:, 0:1], rhs=m4[:], start=True,
                stop=True,
            )
            cntf = spool.tile([1, NTYPES * KS], F32, tag="cntf")
            nc.vector.tensor_copy(out=cntf[:], in_=cntf_p[:])
            cntT = spool.tile([KS, NTYPES], F32, tag="cntT")
            for t in range(NTYPES):
                col_p = sppool.tile([KS, 1], F32, tag="ps")
                nc.tensor.transpose(
                    out=col_p[:], in_=cntf[0:1, t * KS:(t + 1) * KS],
                    identity=ident[0:1, 0:1],
                )
                nc.vector.tensor_copy(out=cntT[:, t:t + 1], in_=col_p[:])

            # ---- totals, bucket offsets, per-tile O ----
            tot_p = sppool.tile([1, NTYPES], F32, tag="ps")
            nc.tensor.matmul(
                out=tot_p[:], lhsT=ones[0:KS, 0:1], rhs=cntT[:], start=True,
                stop=True,
            )
            tot = spool.tile([1, NTYPES], F32, tag="tot")
            nc.vector.tensor_copy(out=tot[:], in_=tot_p[:])
            nc.sync.dma_start(out=cnt_out.ap()[s:s + 1, :], in_=tot[:])
            bofs = spool.tile([1, NTYPES], F32, tag="bofs")
            nc.vector.memset(bofs[0:1, :], 0.0)
            for t in range(1, NTYPES):
                nc.vector.tensor_add(
                    out=bofs[0:1, t:t + 1], in0=bofs[0:1, t - 1:t],
                    in1=tot[0:1, t - 1:t],
                )
            o_kt_p = sppool.tile([KS, NTYPES], F32, tag="ps")
            nc.tensor.matmul(
                out=o_kt_p[:], lhsT=lmat[0:KS, 0:KS], rhs=cntT[:],
                start=True, stop=False,
            )
            nc.tensor.matmul(
                out=o_kt_p[:], lhsT=ones[0:1, 0:KS], rhs=bofs[0:1, :],
                start=False, stop=True,
            )
            o_kt = spool.tile([KS, NTYPES], F32, tag="o_kt")
            nc.vector.tensor_copy(out=o_kt[:], in_=o_kt_p[:])

            # ---- dst[a, k] = sum_t mask_t * (rank_t + O[t, k]) ----
            # each type's PSUM group opens (triangular rank matmul) and
            # closes (rank-1 O broadcast add) immediately, rotating 2 bufs
            dst_ak = spool.tile([P, KS], F32, tag="dst_ak")
            for t in range(NTYPES):
                orow_p = sppool.tile([1, KS], F32, tag="ps")
                nc.tensor.transpose(
                    out=orow_p[:], in_=o_kt[:, t:t + 1], identity=ident[0:KS, 0:KS]
                )
                o_row = spool.tile([1, KS], F32, tag=f"orow{t}")
                nc.vector.tensor_copy(out=o_row[:], in_=orow_p[:])
                rkp = rkpool.tile([P, KS], F32, tag="rank")
                nc.tensor.matmul(
                    out=rkp[:], lhsT=lmat[:], rhs=m4[:, t, :],
                    start=True, stop=False,
                )
                nc.tensor.matmul(
                    out=rkp[:], lhsT=ones[0:1, :], rhs=o_row[0:1, :],
                    start=False, stop=True,
                )
                tmp = spool.tile([P, KS], F32, tag="tmp")
                nc.vector.tensor_tensor(
                    out=tmp[:], in0=rkp[:], in1=m4[:, t, :],
                    op=AluOpType.mult,
                )
                if t == 0:
                    nc.vector.tensor_copy(out=dst_ak[:], in_=tmp[:])
                else:
                    nc.vector.tensor_add(out=dst_ak[:], in0=dst_ak[:], in1=tmp[:])

            # ---- int16 wrapped index tile ----
            # scatter record j = k*128 + a (k within sub-shard); ucode wraps
            # 16-wide: idx[p, j//16] with j//16 = k*8 + a//16, p = a%16, so
            # idx16[p, k, q] = dst_ak[q*16 + p%16, k].
            idx16f = spool.tile([P, KS, 8], F32, tag="idx16f")
            for q in range(8):
                ext_p = sppool.tile([16, KS], F32, tag="ps")
                nc.tensor.matmul(
                    out=ext_p[:], lhsT=ident[:, q * 16:(q + 1) * 16],
                    rhs=dst_ak[:], start=True, stop=True,
                )
                nc.vector.tensor_copy(out=idx16f[0:16, :, q], in_=ext_p[:])
            idx16 = spool.tile([P, KS, 8], I16, tag="idx16")
            nc.vector.tensor_copy(out=idx16[0:16, :, :], in_=idx16f[0:16, :, :])
            for lo, hi in ((16, 32), (32, 64), (64, 128)):
                nc.sync.dma_start(
                    out=idx16[lo:hi, :, :], in_=idx16[lo - (hi - lo):lo, :, :]
                )

            # ---- routing scatter (one instruction, 2048 indices) ----
            nc.gpsimd.dma_scatter_add(
                table.ap()[s * NSS:(s + 1) * NSS, 0:4],
                rec[:, ksl, :],
                idx16[:],
                NSS,
                NSS,
                4,
                elem_step=REC,
            )

            # ---- streaming epilogue for this sub-shard ----
            rb = rbpool.tile([P, KS, 4], F32, tag="rb")
            nc.sync.dma_start(out=rb[:], in_=table_r[:, ksl, 0:4])
            nc.vector.tensor_copy(out=conf_pk[:, ksl], in_=rb[:, :, 3])
            for g4 in range(KS // 4):
                dt_ = descpool.tile([P, 4, NDESC], F32, tag="desc")
                for j in range(4):
                    jj = g4 * 4 + j
                    trp = trppool.tile([4, P], F32, tag="trp")
                    nc.tensor.transpose(
                        out=trp[:], in_=rb[:, jj, :], identity=ident[:]
                    )
                    trs = trspool.tile([P, P], F32R, tag="trs")
                    nc.vector.tensor_copy(out=trs[0:4, :], in_=trp[:])
                    mm = mmpool.tile([P, NDESC], F32, tag="mm")
                    nc.tensor.matmul(
                        out=mm[:], lhsT=trs[0:3, :], rhs=w_r[0:3, :],
                        start=True, stop=True,
                    )
                    nc.scalar.activation(
                        out=dt_[:, j, :], in_=mm[:],
                        func=ActivationFunctionType.Tanh,
                    )
                g = s * (KS // 4) + g4
                nc.sync.dma_start(
                    out=desc_r[:, g * 4:(g + 1) * 4, :], in_=dt_[:]
                )
        nc.sync.dma_start(out=conf_out.ap(), in_=conf_pk[:])

    nc.compile()
    _CACHE["nc"] = nc
    return nc


def make_in_maps(coords, atom_types, W):
    """Shard full inputs into the 8 per-core input maps."""
    coords_flat = np.ascontiguousarray(
        np.asarray(coords, dtype=np.float32).reshape(N, 3)
    )
    types_flat = np.asarray(atom_types).reshape(N).astype(np.int32)
    w_np = np.ascontiguousarray(np.asarray(W, dtype=np.float32))
    lmat = np.triu(np.ones((P, P), dtype=np.float32), k=1)
    ident = np.eye(P, dtype=np.float32)
    conf_all = (np.arange(N, dtype=np.int64) // N_ATOMS).astype(np.float32)

    in_maps = []
    for s in range(N_CORES):
        sl = slice(s * NS, (s + 1) * NS)
        in_maps.append({
            "coords": coords_flat[sl],
            "types": np.ascontiguousarray(types_flat[sl].reshape(KT, P)),
            "confg": np.ascontiguousarray(conf_all[sl].reshape(KT, P)),
            "w_in": w_np,
            "lmat_in": lmat,
            "ident_in": ident,
        })
    return in_maps


def assemble(results):
    """Stitch the 8 cores x SS sub-shards back into full outputs."""
    # counts[c, s, t]
    counts = np.stack([
        np.rint(res["cnt_out"]).astype(np.int64) for res in results
    ])
    at_type_count = counts.sum(axis=(0, 1)).astype(np.int32)

    descs = [res["desc_out"] for res in results]
    confs = [
        np.rint(res["conf_out"].T.reshape(NS)).astype(np.int32)
        for res in results
    ]
    # per-(core, sub-shard) local bucket offsets
    loc_off = np.concatenate(
        [np.zeros((N_CORES, SS, 1), np.int64), np.cumsum(counts, axis=2)],
        axis=2,
    )
    desc_full = np.empty((N, NDESC), dtype=np.float32)
    conf_full = np.empty(N, dtype=np.int32)
    pos = 0
    for t in range(NTYPES):
        for c in range(N_CORES):
            for s in range(SS):
                n = int(counts[c, s, t])
                lo = s * NSS + int(loc_off[c, s, t])
                desc_full[pos:pos + n] = descs[c][lo:lo + n]
                conf_full[pos:pos + n] = confs[c][lo:lo + n]
                pos += n
    assert pos == N
    return desc_full, conf_full, at_type_count


def kernel(coords, atom_types, W):
    nc = build_program()
    in_maps = make_in_maps(coords, atom_types, W)
    res = run_bass_kernel_spmd(nc, in_maps, list(range(N_CORES)))
    return assemble(res.results)


# revision 16
# speedup vs baseline: 1.4374x; 1.1204x over previous
"""Trainium2 Bass kernel for the descriptor-module MoE-routing problem.

Computes, for coords (2048, 64, 3), atom_types (2048, 64), W (3, 384):
    desc_sorted    = tanh(coords.reshape(-1, 3) @ W)[argsort_stable(types)]
    at2conf_sorted = at2conf[argsort_stable(types)]
    at_type_count  = bincount(types, 4)

Sharding: data-parallel over conformers - 8 cores x 256 conformers
(16384 atoms each), and each core further splits its shard into SS=8
independent sub-shards of 16 tiles (2048 atoms). A stable sort by type
decomposes over contiguous index ranges: the global bucket for type t is
the concatenation (in range order) of the per-range buckets, so every
(core, sub-shard) routes independently and the host stitches the 64
bucket lists back together. Sub-sharding lets the Q7 scatter stream of
sub-shard s+1 overlap the compute/write epilogue of sub-shard s.

Numerics: coords and W are split on the host into bf16 hi + lo halves
(x = hi + lo to ~2^-17 relative). The K=3 descriptor matmul runs as
three bf16 matmuls (hi*hi + hi*lo + lo*hi, each product exact in the
f32 PSUM accumulator), giving ~1e-5 error instead of bf16's ~4e-3 -
while running the PE at its full 1 cycle/row bf16 rate (an f32 matmul
is 4x slower, an fp32r one 15x less accurate).

Device algorithm per core:
  1. One batched counting-sort pass over all 8 sub-shards: per-type
     masks, within-tile exclusive ranks (strict-triangular matmul),
     per-sub-shard tile prefix sums (block-diagonal triangular matmul)
     and bucket offsets (selector matmuls). All f32, values < 2048,
     exact. dst[a, k] = sub-shard-local sorted position of every atom.
  2. Routing via the dma_scatter_add Q7 custom instruction (the
     generic vector-indirect DGE path is disabled in this toolchain):
     one 2048-index scatter per sub-shard places 16-byte packed records
     [x_hi y_hi z_hi cq | x_lo y_lo z_lo cr] (8 x bf16) at 256-byte row
     stride into a zeroed table slice. dst is a permutation, so each
     row is written exactly once (CCE-add to zero is exact).
  3. Streaming epilogue per sub-shard: read routed records back,
     bf16 PE-transpose each 128-atom tile (batched 4 tiles per PSUM
     tile + one bulk cast), 3x bf16 matmul with w_hi/w_lo, tanh on
     ScalarE, contiguous write of desc_sorted (25 MB/core stream).
     conf rides along as the exact bf16 pair (q, r) = (conf//64,
     conf%64) and is recombined on the host.
"""

import sys

import numpy as np
import ml_dtypes

try:
    import concourse.bass as bass  # noqa: F401
except ImportError:  # pragma: no cover
    for p in ("/opt/trn_rl_repo", "/root/.axon_site/_ro/trn_rl_repo"):
        if p not in sys.path:
            sys.path.insert(0, p)
    import concourse.bass as bass  # noqa: F401

import concourse.bacc as bacc
import concourse.tile as tile
from concourse import mybir
from concourse.bass_utils import run_bass_kernel_spmd
from concourse.mybir import ActivationFunctionType, AluOpType

N_CORES = 8
N_CONFS = 2048
N_ATOMS = 64
NTYPES = 4
NDESC = 384
N = N_CONFS * N_ATOMS  # 131072
NS = N // N_CORES  # 16384 atoms per core
P = 128
KT = NS // P  # 128 tiles per core
SS = 8  # sub-shards per core (independent sorts)
KS = KT // SS  # 16 tiles per sub-shard
NSS = KS * P  # 2048 atoms per sub-shard
RSTRIDE = 128  # table row stride in bf16 elems (256 B, scatter minimum)
F32 = mybir.dt.float32
BF16 = mybir.dt.bfloat16
I16 = mybir.dt.int16
I32 = mybir.dt.int32
BF = ml_dtypes.bfloat16

_CACHE = {}


def build_program():
    """Build + compile the single-core Bass program (run SPMD on 8 cores)."""
    if "nc" in _CACHE:
        return _CACHE["nc"]

    nc = bacc.Bacc("TRN2", target_bir_lowering=False, debug=False)

    rec_in = nc.dram_tensor("rec_in", [P, KT, 8], BF16, kind="ExternalInput")
    types_in = nc.dram_tensor("types_ak", [P, KT], I32, kind="ExternalInput")
    whi_in = nc.dram_tensor("whi_in", [3, NDESC], BF16, kind="ExternalInput")
    wlo_in = nc.dram_tensor("wlo_in", [3, NDESC], BF16, kind="ExternalInput")
    lmat_in = nc.dram_tensor("lmat_in", [P, P], F32, kind="ExternalInput")
    lblk_in = nc.dram_tensor("lblk_in", [P, P], F32, kind="ExternalInput")
    sel8_in = nc.dram_tensor("sel8_in", [SS, P], F32, kind="ExternalInput")
    selT_in = nc.dram_tensor("selT_in", [P, SS], F32, kind="ExternalInput")
    ident_in = nc.dram_tensor("ident_in", [P, P], F32, kind="ExternalInput")

    table = nc.dram_tensor("table", [NS, RSTRIDE], BF16)  # routed records

    desc_out = nc.dram_tensor("desc_out", [NS, NDESC], F32, kind="ExternalOutput")
    cq_out = nc.dram_tensor("cq_out", [P, KT], F32, kind="ExternalOutput")
    cr_out = nc.dram_tensor("cr_out", [P, KT], F32, kind="ExternalOutput")
    cnt_out = nc.dram_tensor("cnt_out", [SS, NTYPES], F32, kind="ExternalOutput")

    table_z = table.ap().rearrange("(s p f) c -> s p (f c)", s=SS, p=P)
    table_r = table.ap().rearrange("(t p) c -> p t c", p=P)
    desc_r = desc_out.ap().rearrange("(t p) d -> p t d", p=P)

    with tile.TileContext(nc) as tc, \
            tc.tile_pool(name="const", bufs=1) as cpool, \
            tc.tile_pool(name="persist", bufs=1) as ppool, \
            tc.tile_pool(name="setup", bufs=2) as spool, \
            tc.tile_pool(name="spsum", bufs=1, space="PSUM") as sppool, \
            tc.tile_pool(name="rankps", bufs=1, space="PSUM") as rkpool, \
            tc.tile_pool(name="rb", bufs=3) as rbpool, \
            tc.tile_pool(name="trs", bufs=3) as trspool, \
            tc.tile_pool(name="descp", bufs=3) as descpool, \
            tc.tile_pool(name="mm", bufs=4, space="PSUM") as mmpool, \
            tc.tile_pool(name="trp", bufs=2, space="PSUM") as trppool:

        # ---- constants ----
        w_hi = cpool.tile([P, NDESC], BF16, tag="whi")
        nc.sync.dma_start(out=w_hi[0:3, :], in_=whi_in.ap())
        w_lo = cpool.tile([P, NDESC], BF16, tag="wlo")
        nc.sync.dma_start(out=w_lo[0:3, :], in_=wlo_in.ap())
        lmat = cpool.tile([P, P], F32, tag="lmat")
        nc.sync.dma_start(out=lmat[:], in_=lmat_in.ap())
        lblk = cpool.tile([P, P], F32, tag="lblk")
        nc.sync.dma_start(out=lblk[:], in_=lblk_in.ap())
        sel8 = cpool.tile([SS, P], F32, tag="sel8")
        nc.sync.dma_start(out=sel8[:], in_=sel8_in.ap())
        selT = cpool.tile([P, SS], F32, tag="selT")
        nc.sync.dma_start(out=selT[:], in_=selT_in.ap())
        ident = cpool.tile([P, P], F32, tag="ident")
        nc.sync.dma_start(out=ident[:], in_=ident_in.ap())
        ones = cpool.tile([P, P], F32, tag="ones")
        nc.vector.memset(ones[:], 1.0)
        ident_bf = cpool.tile([P, P], BF16, tag="identbf")
        nc.vector.tensor_copy(out=ident_bf[:], in_=ident[:])
        zeros = cpool.tile([P, RSTRIDE * NSS // P], BF16, tag="zeros")
        nc.vector.memset(zeros[:], 0.0)

        # ---- whole-shard loads (host pre-arranged in [a, k] layout) ----
        rec = ppool.tile([P, KT, 8], BF16, tag="rec")
        nc.sync.dma_start(out=rec[:], in_=rec_in.ap())
        t_i = ppool.tile([P, KT], I32, tag="t_i")
        nc.sync.dma_start(out=t_i[:], in_=types_in.ap())
        t_ak = ppool.tile([P, KT], F32, tag="t_ak")
        nc.vector.tensor_copy(out=t_ak[:], in_=t_i[:])

        # ---- per-type masks over the whole shard ----
        m4 = ppool.tile([P, NTYPES, KT], F32, tag="m4")
        for t in range(NTYPES):
            nc.vector.tensor_scalar(
                out=m4[:, t, :], in0=t_ak[:], scalar1=float(t), scalar2=None,
                op0=AluOpType.is_equal,
            )

        # ---- counts cnt[t, k] -> cntT[k, t] (partition 0 aligned) ----
        cntf_p = sppool.tile([1, NTYPES * KT], F32, tag="ps")
        nc.tensor.matmul(
            out=cntf_p[:], lhsT=ones[:, 0:1], rhs=m4[:], start=True, stop=True
        )
        cntf = spool.tile([1, NTYPES * KT], F32, tag="cntf")
        nc.vector.tensor_copy(out=cntf[:], in_=cntf_p[:])
        cntT = spool.tile([P, NTYPES], F32, tag="cntT")
        for t in range(NTYPES):
            col_p = sppool.tile([P, 1], F32, tag="ps")
            nc.tensor.transpose(
                out=col_p[:], in_=cntf[0:1, t * KT:(t + 1) * KT],
                identity=ident[0:1, 0:1],
            )
            nc.vector.tensor_copy(out=cntT[:, t:t + 1], in_=col_p[:])

        # ---- per-sub-shard totals, bucket offsets, per-tile O ----
        tot_p = sppool.tile([SS, NTYPES], F32, tag="ps")
        nc.tensor.matmul(
            out=tot_p[:], lhsT=selT[:], rhs=cntT[:], start=True, stop=True
        )
        tot = spool.tile([SS, NTYPES], F32, tag="tot")
        nc.vector.tensor_copy(out=tot[:], in_=tot_p[:])
        nc.sync.dma_start(out=cnt_out.ap(), in_=tot[:])
        bofs = spool.tile([SS, NTYPES], F32, tag="bofs")
        nc.vector.memset(bofs[:, 0:1], 0.0)
        for t in range(1, NTYPES):
            nc.vector.tensor_add(
                out=bofs[:, t:t + 1], in0=bofs[:, t - 1:t], in1=tot[:, t - 1:t]
            )
        # O[k, t] = bucket_offset[subshard(k), t] + within-sub-shard prefix
        o_kt_p = sppool.tile([P, NTYPES], F32, tag="ps")
        nc.tensor.matmul(
            out=o_kt_p[:], lhsT=lblk[:], rhs=cntT[:], start=True, stop=False
        )
        nc.tensor.matmul(
            out=o_kt_p[:], lhsT=sel8[:], rhs=bofs[:], start=False, stop=True
        )
        o_kt = spool.tile([P, NTYPES], F32, tag="o_kt")
        nc.vector.tensor_copy(out=o_kt[:], in_=o_kt_p[:])

        # ---- dst[a, k] = sum_t mask_t * (rank_t + O[t, k]) ----
        dst_ak = ppool.tile([P, KT], F32, tag="dst_ak")
        for t in range(NTYPES):
            orow_p = sppool.tile([1, P], F32, tag="ps")
            nc.tensor.transpose(
                out=orow_p[:], in_=o_kt[:, t:t + 1], identity=ident[:]
            )
            o_row = spool.tile([1, P], F32, tag=f"orow{t}")
            nc.vector.tensor_copy(out=o_row[:], in_=orow_p[:])
            rkp = rkpool.tile([P, KT], F32, tag="rank")
            nc.tensor.matmul(
                out=rkp[:], lhsT=lmat[:], rhs=m4[:, t, :], start=True, stop=False
            )
            nc.tensor.matmul(
                out=rkp[:], lhsT=ones[0:1, :], rhs=o_row[0:1, :],
                start=False, stop=True,
            )
            tmp = spool.tile([P, KT], F32, tag="tmp")
            nc.vector.tensor_tensor(
                out=tmp[:], in0=rkp[:], in1=m4[:, t, :], op=AluOpType.mult
            )
            if t == 0:
                nc.vector.tensor_copy(out=dst_ak[:], in_=tmp[:])
            else:
                nc.vector.tensor_add(out=dst_ak[:], in0=dst_ak[:], in1=tmp[:])

        # ---- int16 wrapped index tile for all sub-shards ----
        # per sub-shard scatter record j = k*128 + a; the ucode wraps 16
        # wide: idx16[p, k, q] = dst_ak[q*16 + p%16, k] (values are local
        # to each sub-shard already).
        idx16f = ppool.tile([P, KT, 8], F32, tag="idx16f")
        for q in range(8):
            ext_p = sppool.tile([16, P], F32, tag="ps")
            nc.tensor.matmul(
                out=ext_p[:], lhsT=ident[:, q * 16:(q + 1) * 16], rhs=dst_ak[:],
                start=True, stop=True,
            )
            nc.vector.tensor_copy(out=idx16f[0:16, :, q], in_=ext_p[:])
        idx16 = ppool.tile([P, KT, 8], I16, tag="idx16")
        nc.vector.tensor_copy(out=idx16[0:16, :, :], in_=idx16f[0:16, :, :])
        for lo, hi in ((16, 32), (32, 64), (64, 128)):
            nc.sync.dma_start(
                out=idx16[lo:hi, :, :], in_=idx16[lo - (hi - lo):lo, :, :]
            )

        cq_pk = ppool.tile([P, KT], F32, tag="cq_pk")
        cr_pk = ppool.tile([P, KT], F32, tag="cr_pk")

        # ---- per-sub-shard: zero, scatter, epilogue ----
        for s in range(SS):
            ksl = slice(s * KS, (s + 1) * KS)
            nc.sync.dma_start(out=table_z[s], in_=zeros[:])
            nc.gpsimd.dma_scatter_add(
                table.ap()[s * NSS:(s + 1) * NSS, 0:8],
                rec[:, ksl, :],
                idx16[:, ksl, :],
                NSS,
                NSS,
                8,
                elem_step=RSTRIDE,
            )

            rb = rbpool.tile([P, KS, 8], BF16, tag="rb")
            nc.sync.dma_start(out=rb[:], in_=table_r[:, ksl, 0:8])
            nc.vector.tensor_copy(out=cq_pk[:, ksl], in_=rb[:, :, 3])
            nc.vector.tensor_copy(out=cr_pk[:, ksl], in_=rb[:, :, 7])
            for g4 in range(KS // 4):
                trpH = trppool.tile([4, 4, P], BF16, tag="trp")
                trpL = trppool.tile([4, 4, P], BF16, tag="trp")
                for j in range(4):
                    jj = g4 * 4 + j
                    nc.tensor.transpose(
                        out=trpH[:, j, :], in_=rb[:, jj, 0:4],
                        identity=ident_bf[:],
                    )
                    nc.tensor.transpose(
                        out=trpL[:, j, :], in_=rb[:, jj, 4:8],
                        identity=ident_bf[:],
                    )
                trsH = trspool.tile([4, 4, P], BF16, tag="trsH")
                nc.vector.tensor_copy(out=trsH[:], in_=trpH[:])
                trsL = trspool.tile([4, 4, P], BF16, tag="trsL")
                nc.vector.tensor_copy(out=trsL[:], in_=trpL[:])
                dt_ = descpool.tile([P, 4, NDESC], F32, tag="desc")
                for j in range(4):
                    mm = mmpool.tile([P, NDESC], F32, tag="mm")
                    nc.tensor.matmul(
                        out=mm[:], lhsT=trsH[0:3, j, :], rhs=w_hi[0:3, :],
                        start=True, stop=False,
                    )
                    nc.tensor.matmul(
                        out=mm[:], lhsT=trsH[0:3, j, :], rhs=w_lo[0:3, :],
                        start=False, stop=False,
                    )
                    nc.tensor.matmul(
                        out=mm[:], lhsT=trsL[0:3, j, :], rhs=w_hi[0:3, :],
                        start=False, stop=True,
                    )
                    nc.scalar.activation(
                        out=dt_[:, j, :], in_=mm[:],
                        func=ActivationFunctionType.Tanh,
                    )
                g = s * (KS // 4) + g4
                nc.sync.dma_start(
                    out=desc_r[:, g * 4:(g + 1) * 4, :], in_=dt_[:]
                )
        nc.sync.dma_start(out=cq_out.ap(), in_=cq_pk[:])
        nc.sync.dma_start(out=cr_out.ap(), in_=cr_pk[:])

    nc.compile()
    _CACHE["nc"] = nc
    return nc


def make_in_maps(coords, atom_types, W):
    """Shard + pre-pack full inputs into the 8 per-core input maps."""
    coords_flat = np.asarray(coords, dtype=np.float32).reshape(N, 3)
    types_flat = np.asarray(atom_types).reshape(N).astype(np.int32)
    w_np = np.asarray(W, dtype=np.float32)

    w_hi = w_np.astype(BF)
    w_lo = (w_np - w_hi.astype(np.float32)).astype(BF)

    c_hi = coords_flat.astype(BF)
    c_lo = (coords_flat - c_hi.astype(np.float32)).astype(BF)
    conf_g = np.arange(N, dtype=np.int64) // N_ATOMS  # global conformer id
    cq = (conf_g // 64).astype(BF)
    cr = (conf_g % 64).astype(BF)
    # packed record [x_hi y_hi z_hi cq | x_lo y_lo z_lo cr] per atom
    rec_all = np.empty((N, 8), dtype=BF)
    rec_all[:, 0:3] = c_hi
    rec_all[:, 3] = cq
    rec_all[:, 4:7] = c_lo
    rec_all[:, 7] = cr

    lmat = np.triu(np.ones((P, P), dtype=np.float32), k=1)
    kk = np.arange(P)
    lblk = (lmat * (kk[:, None] // KS == kk[None, :] // KS)).astype(np.float32)
    sel8 = (np.arange(SS)[:, None] == kk[None, :] // KS).astype(np.float32)
    selT = np.ascontiguousarray(sel8.T)
    ident = np.eye(P, dtype=np.float32)

    in_maps = []
    for s in range(N_CORES):
        sl = slice(s * NS, (s + 1) * NS)
        # [a, k] layouts: atom i = k*128 + a within the core shard
        rec_ak = np.ascontiguousarray(
            rec_all[sl].reshape(KT, P, 8).transpose(1, 0, 2)
        )
        types_ak = np.ascontiguousarray(types_flat[sl].reshape(KT, P).T)
        in_maps.append({
            "rec_in": rec_ak,
            "types_ak": types_ak,
            "whi_in": np.ascontiguousarray(w_hi),
            "wlo_in": np.ascontiguousarray(w_lo),
            "lmat_in": lmat,
            "lblk_in": lblk,
            "sel8_in": sel8,
            "selT_in": selT,
            "ident_in": ident,
        })
    return in_maps


def assemble(results):
    """Stitch the 8 cores x SS sub-shards back into full outputs."""
    counts = np.stack([
        np.rint(res["cnt_out"]).astype(np.int64) for res in results
    ])  # (cores, SS, types)
    at_type_count = counts.sum(axis=(0, 1)).astype(np.int32)

    descs = [res["desc_out"] for res in results]
    confs = [
        (np.rint(res["cq_out"].T.reshape(NS)).astype(np.int32) * 64
         + np.rint(res["cr_out"].T.reshape(NS)).astype(np.int32))
        for res in results
    ]
    loc_off = np.concatenate(
        [np.zeros((N_CORES, SS, 1), np.int64), np.cumsum(counts, axis=2)],
        axis=2,
    )
    desc_full = np.empty((N, NDESC), dtype=np.float32)
    conf_full = np.empty(N, dtype=np.int32)
    pos = 0
    for t in range(NTYPES):
        for c in range(N_CORES):
            for s in range(SS):
                n = int(counts[c, s, t])
                lo = s * NSS + int(loc_off[c, s, t])
                desc_full[pos:pos + n] = descs[c][lo:lo + n]
                conf_full[pos:pos + n] = confs[c][lo:lo + n]
                pos += n
    assert pos == N
    return desc_full, conf_full, at_type_count


def kernel(coords, atom_types, W):
    nc = build_program()
    in_maps = make_in_maps(coords, atom_types, W)
    res = run_bass_kernel_spmd(nc, in_maps, list(range(N_CORES)))
    return assemble(res.results)


# revision 19
# speedup vs baseline: 1.4930x; 1.0387x over previous
"""Trainium2 Bass kernel for the descriptor-module MoE-routing problem.

Computes, for coords (2048, 64, 3), atom_types (2048, 64), W (3, 384):
    desc_sorted    = tanh(coords.reshape(-1, 3) @ W)[argsort_stable(types)]
    at2conf_sorted = at2conf[argsort_stable(types)]
    at_type_count  = bincount(types, 4)

Sharding: data-parallel over conformers - 8 cores x 256 conformers
(16384 atoms each), and each core further splits its shard into SS=8
independent sub-shards of 16 tiles (2048 atoms). A stable sort by type
decomposes over contiguous index ranges: the global bucket for type t is
the concatenation (in range order) of the per-range buckets, so every
(core, sub-shard) routes independently and the host stitches the 64
bucket lists back together. Sub-sharding lets the Q7 scatter stream of
sub-shard s+1 overlap the compute/write epilogue of sub-shard s.

Numerics: coords and W are split on the host into bf16 hi + lo halves
(x = hi + lo to ~2^-17 relative). The K=3 descriptor matmul runs as
three bf16 matmuls (hi*hi + hi*lo + lo*hi, each product exact in the
f32 PSUM accumulator), giving ~1e-5 error instead of bf16's ~4e-3 -
while running the PE at its full 1 cycle/row bf16 rate (an f32 matmul
is 4x slower, an fp32r one 15x less accurate).

Device algorithm per core:
  1. One batched counting-sort pass over all 8 sub-shards: per-type
     masks, within-tile exclusive ranks (strict-triangular matmul),
     per-sub-shard tile prefix sums (block-diagonal triangular matmul)
     and bucket offsets (selector matmuls). All f32, values < 2048,
     exact. dst[a, k] = sub-shard-local sorted position of every atom.
  2. Routing via the dma_scatter_add Q7 custom instruction (the
     generic vector-indirect DGE path is disabled in this toolchain):
     one 2048-index scatter per sub-shard places 16-byte packed records
     [x_hi y_hi z_hi cq | x_lo y_lo z_lo cr] (8 x bf16) at 256-byte row
     stride into a zeroed table slice. dst is a permutation, so each
     row is written exactly once (CCE-add to zero is exact).
  3. Streaming epilogue per sub-shard: read routed records back,
     bf16 PE-transpose each 128-atom tile (batched 4 tiles per PSUM
     tile + one bulk cast), 3x bf16 matmul with w_hi/w_lo, tanh on
     ScalarE, contiguous write of desc_sorted (25 MB/core stream).
     conf rides along as the exact bf16 pair (q, r) = (conf//64,
     conf%64) and is recombined on the host.
"""

import sys

import numpy as np
import ml_dtypes

try:
    import concourse.bass as bass  # noqa: F401
except ImportError:  # pragma: no cover
    for p in ("/opt/trn_rl_repo", "/root/.axon_site/_ro/trn_rl_repo"):
        if p not in sys.path:
            sys.path.insert(0, p)
    import concourse.bass as bass  # noqa: F401

import concourse.bacc as bacc
import concourse.tile as tile
from concourse import mybir
from concourse.bass_utils import run_bass_kernel_spmd
from concourse.mybir import ActivationFunctionType, AluOpType

N_CORES = 8
N_CONFS = 2048
N_ATOMS = 64
NTYPES = 4
NDESC = 384
N = N_CONFS * N_ATOMS  # 131072
NS = N // N_CORES  # 16384 atoms per core
P = 128
KT = NS // P  # 128 tiles per core
SS = 8  # sub-shards per core (independent sorts)
KS = KT // SS  # 16 tiles per sub-shard
NSS = KS * P  # 2048 atoms per sub-shard
RSTRIDE = 128  # table row stride in bf16 elems (256 B, scatter minimum)
F32 = mybir.dt.float32
BF16 = mybir.dt.bfloat16
I16 = mybir.dt.int16
I32 = mybir.dt.int32
BF = ml_dtypes.bfloat16

_CACHE = {}


def build_program():
    """Build + compile the single-core Bass program (run SPMD on 8 cores)."""
    if "nc" in _CACHE:
        return _CACHE["nc"]

    nc = bacc.Bacc("TRN2", target_bir_lowering=False, debug=False)

    rec_in = nc.dram_tensor("rec_in", [P, KT, 8], BF16, kind="ExternalInput")
    types_in = nc.dram_tensor("types_ak", [P, KT], I32, kind="ExternalInput")
    w6_in = nc.dram_tensor("w6_in", [6, NDESC], BF16, kind="ExternalInput")
    wlo_in = nc.dram_tensor("wlo_in", [3, NDESC], BF16, kind="ExternalInput")
    lmat_in = nc.dram_tensor("lmat_in", [P, P], F32, kind="ExternalInput")
    lblk_in = nc.dram_tensor("lblk_in", [P, P], F32, kind="ExternalInput")
    sel8_in = nc.dram_tensor("sel8_in", [SS, P], F32, kind="ExternalInput")
    selT_in = nc.dram_tensor("selT_in", [P, SS], F32, kind="ExternalInput")
    ident_in = nc.dram_tensor("ident_in", [P, P], F32, kind="ExternalInput")

    table = nc.dram_tensor("table", [NS, RSTRIDE], BF16)  # routed records

    desc_out = nc.dram_tensor("desc_out", [NS, NDESC], F32, kind="ExternalOutput")
    cqr_out = nc.dram_tensor("cqr_out", [P, KT, 2], BF16, kind="ExternalOutput")
    cnt_out = nc.dram_tensor("cnt_out", [SS, NTYPES], F32, kind="ExternalOutput")

    table_z = table.ap().rearrange("(s p f) c -> s p (f c)", s=SS, p=P)
    table_r = table.ap().rearrange("(t p) c -> p t c", p=P)
    desc_r = desc_out.ap().rearrange("(t p) d -> p t d", p=P)

    with tile.TileContext(nc) as tc, \
            tc.tile_pool(name="const", bufs=1) as cpool, \
            tc.tile_pool(name="persist", bufs=1) as ppool, \
            tc.tile_pool(name="setup", bufs=2) as spool, \
            tc.tile_pool(name="rb", bufs=3) as rbpool, \
            tc.tile_pool(name="trs", bufs=4) as trspool, \
            tc.tile_pool(name="descp", bufs=3) as descpool:
        setup_psum = tc.tile_pool(name="spsum", bufs=3, space="PSUM")
        sppool = setup_psum.__enter__()
        rank_psum = tc.tile_pool(name="rankps", bufs=2, space="PSUM")
        rkpool = rank_psum.__enter__()

        # ---- constants ----
        w6 = cpool.tile([P, NDESC], BF16, tag="w6")
        nc.scalar.dma_start(out=w6[0:6, :], in_=w6_in.ap())
        w_lo = cpool.tile([P, NDESC], BF16, tag="wlo")
        nc.scalar.dma_start(out=w_lo[0:3, :], in_=wlo_in.ap())
        lmat = cpool.tile([P, P], F32, tag="lmat")
        nc.sync.dma_start(out=lmat[:], in_=lmat_in.ap())
        lblk = cpool.tile([P, P], F32, tag="lblk")
        nc.scalar.dma_start(out=lblk[:], in_=lblk_in.ap())
        sel8 = cpool.tile([SS, P], F32, tag="sel8")
        nc.scalar.dma_start(out=sel8[:], in_=sel8_in.ap())
        selT = cpool.tile([P, SS], F32, tag="selT")
        nc.scalar.dma_start(out=selT[:], in_=selT_in.ap())
        ident = cpool.tile([P, P], F32, tag="ident")
        nc.sync.dma_start(out=ident[:], in_=ident_in.ap())
        ones = cpool.tile([P, P], F32, tag="ones")
        nc.vector.memset(ones[:], 1.0)
        ident_bf = cpool.tile([P, P], BF16, tag="identbf")
        nc.vector.tensor_copy(out=ident_bf[:], in_=ident[:])
        zeros = cpool.tile([P, RSTRIDE * NSS // P], BF16, tag="zeros")
        nc.vector.memset(zeros[:], 0.0)

        # ---- whole-shard loads (host pre-arranged in [a, k] layout) ----
        rec = ppool.tile([P, KT, 8], BF16, tag="rec")
        nc.sync.dma_start(out=rec[:], in_=rec_in.ap())
        t_i = ppool.tile([P, KT], I32, tag="t_i")
        nc.sync.dma_start(out=t_i[:], in_=types_in.ap())
        t_ak = ppool.tile([P, KT], F32, tag="t_ak")
        nc.vector.tensor_copy(out=t_ak[:], in_=t_i[:])

        # ---- per-type masks over the whole shard ----
        m4 = ppool.tile([P, NTYPES, KT], F32, tag="m4")
        for t in range(NTYPES):
            nc.vector.tensor_scalar(
                out=m4[:, t, :], in0=t_ak[:], scalar1=float(t), scalar2=None,
                op0=AluOpType.is_equal,
            )

        # ---- counts cnt[t, k] -> cntT[k, t] (partition 0 aligned) ----
        cntf_p = sppool.tile([1, NTYPES * KT], F32, tag="ps")
        nc.tensor.matmul(
            out=cntf_p[:], lhsT=ones[:, 0:1], rhs=m4[:], start=True, stop=True
        )
        cntf = spool.tile([1, NTYPES * KT], F32, tag="cntf")
        nc.vector.tensor_copy(out=cntf[:], in_=cntf_p[:])
        cntT = spool.tile([P, NTYPES], F32, tag="cntT")
        for t in range(NTYPES):
            col_p = sppool.tile([P, 1], F32, tag="ps")
            nc.tensor.transpose(
                out=col_p[:], in_=cntf[0:1, t * KT:(t + 1) * KT],
                identity=ident[0:1, 0:1],
            )
            nc.vector.tensor_copy(out=cntT[:, t:t + 1], in_=col_p[:])

        # ---- per-sub-shard totals, bucket offsets, per-tile O ----
        tot_p = sppool.tile([SS, NTYPES], F32, tag="ps")
        nc.tensor.matmul(
            out=tot_p[:], lhsT=selT[:], rhs=cntT[:], start=True, stop=True
        )
        tot = spool.tile([SS, NTYPES], F32, tag="tot")
        nc.vector.tensor_copy(out=tot[:], in_=tot_p[:])
        nc.sync.dma_start(out=cnt_out.ap(), in_=tot[:])
        bofs = spool.tile([SS, NTYPES], F32, tag="bofs")
        nc.vector.memset(bofs[:, 0:1], 0.0)
        for t in range(1, NTYPES):
            nc.vector.tensor_add(
                out=bofs[:, t:t + 1], in0=bofs[:, t - 1:t], in1=tot[:, t - 1:t]
            )
        # O[k, t] = bucket_offset[subshard(k), t] + within-sub-shard prefix
        o_kt_p = sppool.tile([P, NTYPES], F32, tag="ps")
        nc.tensor.matmul(
            out=o_kt_p[:], lhsT=lblk[:], rhs=cntT[:], start=True, stop=False
        )
        nc.tensor.matmul(
            out=o_kt_p[:], lhsT=sel8[:], rhs=bofs[:], start=False, stop=True
        )
        o_kt = spool.tile([P, NTYPES], F32, tag="o_kt")
        nc.vector.tensor_copy(out=o_kt[:], in_=o_kt_p[:])

        # ---- dst[a, k] = sum_t mask_t * (rank_t + O[t, k]) ----
        dst_ak = ppool.tile([P, KT], F32, tag="dst_ak")
        for t in range(NTYPES):
            orow_p = sppool.tile([1, P], F32, tag="ps")
            nc.tensor.transpose(
                out=orow_p[:], in_=o_kt[:, t:t + 1], identity=ident[:]
            )
            o_row = spool.tile([1, P], F32, tag=f"orow{t}")
            nc.vector.tensor_copy(out=o_row[:], in_=orow_p[:])
            rkp = rkpool.tile([P, KT], F32, tag="rank")
            nc.tensor.matmul(
                out=rkp[:], lhsT=lmat[:], rhs=m4[:, t, :], start=True, stop=False
            )
            nc.tensor.matmul(
                out=rkp[:], lhsT=ones[0:1, :], rhs=o_row[0:1, :],
                start=False, stop=True,
            )
            tmp = spool.tile([P, KT], F32, tag="tmp")
            nc.vector.tensor_tensor(
                out=tmp[:], in0=rkp[:], in1=m4[:, t, :], op=AluOpType.mult
            )
            if t == 0:
                nc.vector.tensor_copy(out=dst_ak[:], in_=tmp[:])
            else:
                nc.vector.tensor_add(out=dst_ak[:], in0=dst_ak[:], in1=tmp[:])

        # ---- int16 wrapped index tile for all sub-shards ----
        # per sub-shard scatter record j = k*128 + a; the ucode wraps 16
        # wide: idx16[p, k, q] = dst_ak[q*16 + p%16, k] (values are local
        # to each sub-shard already).
        idx16f = ppool.tile([P, KT, 8], F32, tag="idx16f")
        for q in range(8):
            ext_p = sppool.tile([16, P], F32, tag="ps")
            nc.tensor.matmul(
                out=ext_p[:], lhsT=ident[:, q * 16:(q + 1) * 16], rhs=dst_ak[:],
                start=True, stop=True,
            )
            nc.vector.tensor_copy(out=idx16f[0:16, :, q], in_=ext_p[:])
        idx16 = ppool.tile([P, KT, 8], I16, tag="idx16")
        nc.vector.tensor_copy(out=idx16[0:16, :, :], in_=idx16f[0:16, :, :])
        for lo, hi in ((16, 32), (32, 64), (64, 128)):
            nc.sync.dma_start(
                out=idx16[lo:hi, :, :], in_=idx16[lo - (hi - lo):lo, :, :]
            )

        rank_psum.__exit__(None, None, None)
        setup_psum.__exit__(None, None, None)
        mm_psum = tc.tile_pool(name="mm", bufs=4, space="PSUM")
        mmpool = mm_psum.__enter__()
        trp_psum = tc.tile_pool(name="trp", bufs=3, space="PSUM")
        trppool = trp_psum.__enter__()

        # ---- per-sub-shard: zero, scatter, epilogue ----
        for s in range(SS):
            ksl = slice(s * KS, (s + 1) * KS)
            nc.sync.dma_start(out=table_z[s], in_=zeros[:])
            nc.gpsimd.dma_scatter_add(
                table.ap()[s * NSS:(s + 1) * NSS, 0:8],
                rec[:, ksl, :],
                idx16[:, ksl, :],
                NSS,
                NSS,
                8,
                elem_step=RSTRIDE,
            )

            rb = rbpool.tile([P, KS, 8], BF16, tag="rb")
            nc.scalar.dma_start(out=rb[:], in_=table_r[:, ksl, 0:8])
            for g4 in range(KS // 4):
                trp = trppool.tile([8, 4, P], BF16, tag="trp")
                for j in range(4):
                    jj = g4 * 4 + j
                    nc.tensor.transpose(
                        out=trp[:, j, :], in_=rb[:, jj, :],
                        identity=ident_bf[:],
                    )
                trs = trspool.tile([8, 4, P], BF16, tag="trs")
                nc.vector.tensor_copy(out=trs[:], in_=trp[:])
                dt_ = descpool.tile([P, 4, NDESC], F32, tag="desc")
                for j in range(4):
                    mm = mmpool.tile([P, NDESC], F32, tag="mm")
                    # (c_hi + c_lo) @ w_hi in one K=6 matmul
                    nc.tensor.matmul(
                        out=mm[:], lhsT=trs[0:6, j, :], rhs=w6[0:6, :],
                        start=True, stop=False,
                    )
                    # + c_hi @ w_lo
                    nc.tensor.matmul(
                        out=mm[:], lhsT=trs[0:3, j, :], rhs=w_lo[0:3, :],
                        start=False, stop=True,
                    )
                    nc.scalar.activation(
                        out=dt_[:, j, :], in_=mm[:],
                        func=ActivationFunctionType.Tanh,
                    )
                g = s * (KS // 4) + g4
                nc.sync.dma_start(
                    out=desc_r[:, g * 4:(g + 1) * 4, :], in_=dt_[:]
                )
        # conf rides in record slots 6:8; one strided gather at the end
        cqr = ppool.tile([P, KT, 2], BF16, tag="cqr")
        nc.scalar.dma_start(out=cqr[:], in_=table_r[:, :, 6:8])
        nc.sync.dma_start(out=cqr_out.ap(), in_=cqr[:])
        trp_psum.__exit__(None, None, None)
        mm_psum.__exit__(None, None, None)

    nc.compile()
    _CACHE["nc"] = nc
    return nc


def make_in_maps(coords, atom_types, W):
    """Shard + pre-pack full inputs into the 8 per-core input maps."""
    coords_flat = np.asarray(coords, dtype=np.float32).reshape(N, 3)
    types_flat = np.asarray(atom_types).reshape(N).astype(np.int32)
    w_np = np.asarray(W, dtype=np.float32)

    w_hi = w_np.astype(BF)
    w_lo = (w_np - w_hi.astype(np.float32)).astype(BF)

    c_hi = coords_flat.astype(BF)
    c_lo = (coords_flat - c_hi.astype(np.float32)).astype(BF)
    conf_g = np.arange(N, dtype=np.int64) // N_ATOMS  # global conformer id
    cq = (conf_g // 64).astype(BF)
    cr = (conf_g % 64).astype(BF)
    # packed record [x_hi y_hi z_hi x_lo y_lo z_lo cq cr] per atom
    rec_all = np.empty((N, 8), dtype=BF)
    rec_all[:, 0:3] = c_hi
    rec_all[:, 3:6] = c_lo
    rec_all[:, 6] = cq
    rec_all[:, 7] = cr

    lmat = np.triu(np.ones((P, P), dtype=np.float32), k=1)
    kk = np.arange(P)
    lblk = (lmat * (kk[:, None] // KS == kk[None, :] // KS)).astype(np.float32)
    sel8 = (np.arange(SS)[:, None] == kk[None, :] // KS).astype(np.float32)
    selT = np.ascontiguousarray(sel8.T)
    ident = np.eye(P, dtype=np.float32)

    in_maps = []
    for s in range(N_CORES):
        sl = slice(s * NS, (s + 1) * NS)
        # [a, k] layouts: atom i = k*128 + a within the core shard
        rec_ak = np.ascontiguousarray(
            rec_all[sl].reshape(KT, P, 8).transpose(1, 0, 2)
        )
        types_ak = np.ascontiguousarray(types_flat[sl].reshape(KT, P).T)
        in_maps.append({
            "rec_in": rec_ak,
            "types_ak": types_ak,
            "w6_in": np.ascontiguousarray(np.concatenate([w_hi, w_hi], axis=0)),
            "wlo_in": np.ascontiguousarray(w_lo),
            "lmat_in": lmat,
            "lblk_in": lblk,
            "sel8_in": sel8,
            "selT_in": selT,
            "ident_in": ident,
        })
    return in_maps


def assemble(results):
    """Stitch the 8 cores x SS sub-shards back into full outputs."""
    counts = np.stack([
        np.rint(res["cnt_out"]).astype(np.int64) for res in results
    ])  # (cores, SS, types)
    at_type_count = counts.sum(axis=(0, 1)).astype(np.int32)

    descs = [res["desc_out"] for res in results]
    confs = [
        (res["cqr_out"][:, :, 0].T.reshape(NS).astype(np.int32) * 64
         + res["cqr_out"][:, :, 1].T.reshape(NS).astype(np.int32))
        for res in results
    ]
    loc_off = np.concatenate(
        [np.zeros((N_CORES, SS, 1), np.int64), np.cumsum(counts, axis=2)],
        axis=2,
    )
    desc_full = np.empty((N, NDESC), dtype=np.float32)
    conf_full = np.empty(N, dtype=np.int32)
    pos = 0
    for t in range(NTYPES):
        for c in range(N_CORES):
            for s in range(SS):
                n = int(counts[c, s, t])
                lo = s * NSS + int(loc_off[c, s, t])
                desc_full[pos:pos + n] = descs[c][lo:lo + n]
                conf_full[pos:pos + n] = confs[c][lo:lo + n]
                pos += n
    assert pos == N
    return desc_full, conf_full, at_type_count


def kernel(coords, atom_types, W):
    nc = build_program()
    in_maps = make_in_maps(coords, atom_types, W)
    res = run_bass_kernel_spmd(nc, in_maps, list(range(N_CORES)))
    return assemble(res.results)


# revision 20
# speedup vs baseline: 1.4972x; 1.0028x over previous
"""Trainium2 Bass kernel for the descriptor-module MoE-routing problem.

Computes, for coords (2048, 64, 3), atom_types (2048, 64), W (3, 384):
    desc_sorted    = tanh(coords.reshape(-1, 3) @ W)[argsort_stable(types)]
    at2conf_sorted = at2conf[argsort_stable(types)]
    at_type_count  = bincount(types, 4)

Sharding: data-parallel over conformers - 8 cores x 256 conformers
(16384 atoms each), and each core further splits its shard into SS=8
independent sub-shards of 16 tiles (2048 atoms). A stable sort by type
decomposes over contiguous index ranges: the global bucket for type t is
the concatenation (in range order) of the per-range buckets, so every
(core, sub-shard) routes independently and the host stitches the 64
bucket lists back together. Sub-sharding lets the Q7 scatter stream of
sub-shard s+1 overlap the compute/write epilogue of sub-shard s.

Numerics: coords and W are split on the host into bf16 hi + lo halves
(x = hi + lo to ~2^-17 relative). The K=3 descriptor matmul runs as
three bf16 matmuls (hi*hi + hi*lo + lo*hi, each product exact in the
f32 PSUM accumulator), giving ~1e-5 error instead of bf16's ~4e-3 -
while running the PE at its full 1 cycle/row bf16 rate (an f32 matmul
is 4x slower, an fp32r one 15x less accurate).

Device algorithm per core:
  1. One batched counting-sort pass over all 8 sub-shards: per-type
     masks, within-tile exclusive ranks (strict-triangular matmul),
     per-sub-shard tile prefix sums (block-diagonal triangular matmul)
     and bucket offsets (selector matmuls). All f32, values < 2048,
     exact. dst[a, k] = sub-shard-local sorted position of every atom.
  2. Routing via the dma_scatter_add Q7 custom instruction (the
     generic vector-indirect DGE path is disabled in this toolchain):
     one 2048-index scatter per sub-shard places 16-byte packed records
     [x_hi y_hi z_hi cq | x_lo y_lo z_lo cr] (8 x bf16) at 256-byte row
     stride into a zeroed table slice. dst is a permutation, so each
     row is written exactly once (CCE-add to zero is exact).
  3. Streaming epilogue per sub-shard: read routed records back,
     bf16 PE-transpose each 128-atom tile (batched 4 tiles per PSUM
     tile + one bulk cast), 3x bf16 matmul with w_hi/w_lo, tanh on
     ScalarE, contiguous write of desc_sorted (25 MB/core stream).
     conf rides along as the exact bf16 pair (q, r) = (conf//64,
     conf%64) and is recombined on the host.
"""

import sys

import numpy as np
import ml_dtypes

try:
    import concourse.bass as bass  # noqa: F401
except ImportError:  # pragma: no cover
    for p in ("/opt/trn_rl_repo", "/root/.axon_site/_ro/trn_rl_repo"):
        if p not in sys.path:
            sys.path.insert(0, p)
    import concourse.bass as bass  # noqa: F401

import concourse.bacc as bacc
import concourse.tile as tile
from concourse import mybir
from concourse.bass_utils import run_bass_kernel_spmd
from concourse.mybir import ActivationFunctionType, AluOpType

N_CORES = 8
N_CONFS = 2048
N_ATOMS = 64
NTYPES = 4
NDESC = 384
N = N_CONFS * N_ATOMS  # 131072
NS = N // N_CORES  # 16384 atoms per core
P = 128
KT = NS // P  # 128 tiles per core
SS = 8  # sub-shards per core (independent sorts)
KS = KT // SS  # 16 tiles per sub-shard
NSS = KS * P  # 2048 atoms per sub-shard
RSTRIDE = 128  # table row stride in bf16 elems (256 B, scatter minimum)
F32 = mybir.dt.float32
BF16 = mybir.dt.bfloat16
I16 = mybir.dt.int16
I32 = mybir.dt.int32
BF = ml_dtypes.bfloat16

_CACHE = {}


def build_program():
    """Build + compile the single-core Bass program (run SPMD on 8 cores)."""
    if "nc" in _CACHE:
        return _CACHE["nc"]

    nc = bacc.Bacc("TRN2", target_bir_lowering=False, debug=False)

    rec_in = nc.dram_tensor("rec_in", [P, KT, 8], BF16, kind="ExternalInput")
    types_in = nc.dram_tensor("types_ak", [P, KT], I32, kind="ExternalInput")
    w6_in = nc.dram_tensor("w6_in", [6, NDESC], BF16, kind="ExternalInput")
    wlo_in = nc.dram_tensor("wlo_in", [3, NDESC], BF16, kind="ExternalInput")
    lmat_in = nc.dram_tensor("lmat_in", [P, P], F32, kind="ExternalInput")
    lblk_in = nc.dram_tensor("lblk_in", [P, P], F32, kind="ExternalInput")
    sel8_in = nc.dram_tensor("sel8_in", [SS, P], F32, kind="ExternalInput")
    selT_in = nc.dram_tensor("selT_in", [P, SS], F32, kind="ExternalInput")
    ident_in = nc.dram_tensor("ident_in", [P, P], F32, kind="ExternalInput")

    table = nc.dram_tensor("table", [NS, RSTRIDE], BF16)  # routed records

    desc_out = nc.dram_tensor("desc_out", [NS, NDESC], F32, kind="ExternalOutput")
    cqr_out = nc.dram_tensor("cqr_out", [P, KT, 2], BF16, kind="ExternalOutput")
    cnt_out = nc.dram_tensor("cnt_out", [SS, NTYPES], F32, kind="ExternalOutput")

    table_z = table.ap().rearrange("(s p f) c -> s p (f c)", s=SS, p=P)
    table_r = table.ap().rearrange("(t p) c -> p t c", p=P)
    desc_r = desc_out.ap().rearrange("(t p) d -> p t d", p=P)

    with tile.TileContext(nc) as tc, \
            tc.tile_pool(name="const", bufs=1) as cpool, \
            tc.tile_pool(name="persist", bufs=1) as ppool, \
            tc.tile_pool(name="setup", bufs=2) as spool, \
            tc.tile_pool(name="rb", bufs=3) as rbpool, \
            tc.tile_pool(name="trs", bufs=4) as trspool, \
            tc.tile_pool(name="descp", bufs=3) as descpool:
        setup_psum = tc.tile_pool(name="spsum", bufs=3, space="PSUM")
        sppool = setup_psum.__enter__()
        rank_psum = tc.tile_pool(name="rankps", bufs=2, space="PSUM")
        rkpool = rank_psum.__enter__()

        # ---- constants ----
        w6 = cpool.tile([P, NDESC], BF16, tag="w6")
        nc.scalar.dma_start(out=w6[0:6, :], in_=w6_in.ap())
        w_lo = cpool.tile([P, NDESC], BF16, tag="wlo")
        nc.scalar.dma_start(out=w_lo[0:3, :], in_=wlo_in.ap())
        lmat = cpool.tile([P, P], F32, tag="lmat")
        nc.sync.dma_start(out=lmat[:], in_=lmat_in.ap())
        lblk = cpool.tile([P, P], F32, tag="lblk")
        nc.scalar.dma_start(out=lblk[:], in_=lblk_in.ap())
        sel8 = cpool.tile([SS, P], F32, tag="sel8")
        nc.scalar.dma_start(out=sel8[:], in_=sel8_in.ap())
        selT = cpool.tile([P, SS], F32, tag="selT")
        nc.scalar.dma_start(out=selT[:], in_=selT_in.ap())
        ident = cpool.tile([P, P], F32, tag="ident")
        nc.sync.dma_start(out=ident[:], in_=ident_in.ap())
        ones = cpool.tile([P, P], F32, tag="ones")
        nc.vector.memset(ones[:], 1.0)
        ident_bf = cpool.tile([P, P], BF16, tag="identbf")
        nc.vector.tensor_copy(out=ident_bf[:], in_=ident[:])
        zeros = cpool.tile([P, RSTRIDE * NSS // P], BF16, tag="zeros")
        nc.vector.memset(zeros[:], 0.0)

        # ---- whole-shard loads (host pre-arranged in [a, k] layout) ----
        rec = ppool.tile([P, KT, 8], BF16, tag="rec")
        nc.sync.dma_start(out=rec[:], in_=rec_in.ap())
        t_i = ppool.tile([P, KT], I32, tag="t_i")
        nc.sync.dma_start(out=t_i[:], in_=types_in.ap())
        t_ak = ppool.tile([P, KT], F32, tag="t_ak")
        nc.vector.tensor_copy(out=t_ak[:], in_=t_i[:])

        # ---- per-type masks over the whole shard ----
        m4 = ppool.tile([P, NTYPES, KT], F32, tag="m4")
        for t in range(NTYPES):
            nc.vector.tensor_scalar(
                out=m4[:, t, :], in0=t_ak[:], scalar1=float(t), scalar2=None,
                op0=AluOpType.is_equal,
            )

        # ---- counts cnt[t, k] -> cntT[k, t] (partition 0 aligned) ----
        cntf_p = sppool.tile([1, NTYPES * KT], F32, tag="ps")
        nc.tensor.matmul(
            out=cntf_p[:], lhsT=ones[:, 0:1], rhs=m4[:], start=True, stop=True
        )
        cntf = spool.tile([1, NTYPES * KT], F32, tag="cntf")
        nc.vector.tensor_copy(out=cntf[:], in_=cntf_p[:])
        cntT = spool.tile([P, NTYPES], F32, tag="cntT")
        for t in range(NTYPES):
            col_p = sppool.tile([P, 1], F32, tag="ps")
            nc.tensor.transpose(
                out=col_p[:], in_=cntf[0:1, t * KT:(t + 1) * KT],
                identity=ident[0:1, 0:1],
            )
            nc.vector.tensor_copy(out=cntT[:, t:t + 1], in_=col_p[:])

        # ---- per-sub-shard totals, bucket offsets, per-tile O ----
        tot_p = sppool.tile([SS, NTYPES], F32, tag="ps")
        nc.tensor.matmul(
            out=tot_p[:], lhsT=selT[:], rhs=cntT[:], start=True, stop=True
        )
        tot = spool.tile([SS, NTYPES], F32, tag="tot")
        nc.vector.tensor_copy(out=tot[:], in_=tot_p[:])
        nc.sync.dma_start(out=cnt_out.ap(), in_=tot[:])
        bofs = spool.tile([SS, NTYPES], F32, tag="bofs")
        nc.vector.memset(bofs[:, 0:1], 0.0)
        for t in range(1, NTYPES):
            nc.vector.tensor_add(
                out=bofs[:, t:t + 1], in0=bofs[:, t - 1:t], in1=tot[:, t - 1:t]
            )
        # O[k, t] = bucket_offset[subshard(k), t] + within-sub-shard prefix
        o_kt_p = sppool.tile([P, NTYPES], F32, tag="ps")
        nc.tensor.matmul(
            out=o_kt_p[:], lhsT=lblk[:], rhs=cntT[:], start=True, stop=False
        )
        nc.tensor.matmul(
            out=o_kt_p[:], lhsT=sel8[:], rhs=bofs[:], start=False, stop=True
        )
        o_kt = spool.tile([P, NTYPES], F32, tag="o_kt")
        nc.vector.tensor_copy(out=o_kt[:], in_=o_kt_p[:])

        # ---- dst[a, k] = sum_t mask_t * (rank_t + O[t, k]) ----
        dst_ak = ppool.tile([P, KT], F32, tag="dst_ak")
        for t in range(NTYPES):
            orow_p = sppool.tile([1, P], F32, tag="ps")
            nc.tensor.transpose(
                out=orow_p[:], in_=o_kt[:, t:t + 1], identity=ident[:]
            )
            o_row = spool.tile([1, P], F32, tag=f"orow{t}")
            nc.vector.tensor_copy(out=o_row[:], in_=orow_p[:])
            rkp = rkpool.tile([P, KT], F32, tag="rank")
            nc.tensor.matmul(
                out=rkp[:], lhsT=lmat[:], rhs=m4[:, t, :], start=True, stop=False
            )
            nc.tensor.matmul(
                out=rkp[:], lhsT=ones[0:1, :], rhs=o_row[0:1, :],
                start=False, stop=True,
            )
            tmp = spool.tile([P, KT], F32, tag="tmp")
            nc.vector.tensor_tensor(
                out=tmp[:], in0=rkp[:], in1=m4[:, t, :], op=AluOpType.mult
            )
            if t == 0:
                nc.vector.tensor_copy(out=dst_ak[:], in_=tmp[:])
            else:
                nc.vector.tensor_add(out=dst_ak[:], in0=dst_ak[:], in1=tmp[:])

        # ---- int16 wrapped index tile for all sub-shards ----
        # per sub-shard scatter record j = k*128 + a; the ucode wraps 16
        # wide: idx16[p, k, q] = dst_ak[q*16 + p%16, k] (values are local
        # to each sub-shard already).
        idx16f = ppool.tile([P, KT, 8], F32, tag="idx16f")
        for q in range(8):
            ext_p = sppool.tile([16, P], F32, tag="ps")
            nc.tensor.matmul(
                out=ext_p[:], lhsT=ident[:, q * 16:(q + 1) * 16], rhs=dst_ak[:],
                start=True, stop=True,
            )
            nc.vector.tensor_copy(out=idx16f[0:16, :, q], in_=ext_p[:])
        idx16 = ppool.tile([P, KT, 8], I16, tag="idx16")
        nc.vector.tensor_copy(out=idx16[0:16, :, :], in_=idx16f[0:16, :, :])
        for lo, hi in ((16, 32), (32, 64), (64, 128)):
            nc.sync.dma_start(
                out=idx16[lo:hi, :, :], in_=idx16[lo - (hi - lo):lo, :, :]
            )

        rank_psum.__exit__(None, None, None)
        setup_psum.__exit__(None, None, None)
        mm_psum = tc.tile_pool(name="mm", bufs=4, space="PSUM")
        mmpool = mm_psum.__enter__()
        trp_psum = tc.tile_pool(name="trp", bufs=3, space="PSUM")
        trppool = trp_psum.__enter__()

        # ---- per-sub-shard: zero, scatter, epilogue ----
        def emit_mms(trs, g):
            dt_ = descpool.tile([P, 4, NDESC], F32, tag="desc")
            for j in range(4):
                mm = mmpool.tile([P, NDESC], F32, tag="mm")
                # (c_hi + c_lo) @ w_hi in one K=6 matmul
                nc.tensor.matmul(
                    out=mm[:], lhsT=trs[0:6, j, :], rhs=w6[0:6, :],
                    start=True, stop=False,
                )
                # + c_hi @ w_lo
                nc.tensor.matmul(
                    out=mm[:], lhsT=trs[0:3, j, :], rhs=w_lo[0:3, :],
                    start=False, stop=True,
                )
                nc.scalar.activation(
                    out=dt_[:, j, :], in_=mm[:],
                    func=ActivationFunctionType.Tanh,
                )
            nc.sync.dma_start(
                out=desc_r[:, g * 4:(g + 1) * 4, :], in_=dt_[:]
            )

        pend = None
        for s in range(SS):
            ksl = slice(s * KS, (s + 1) * KS)
            nc.sync.dma_start(out=table_z[s], in_=zeros[:])
            nc.gpsimd.dma_scatter_add(
                table.ap()[s * NSS:(s + 1) * NSS, 0:8],
                rec[:, ksl, :],
                idx16[:, ksl, :],
                NSS,
                NSS,
                8,
                elem_step=RSTRIDE,
            )

            rb = rbpool.tile([P, KS, 8], BF16, tag="rb")
            nc.scalar.dma_start(out=rb[:], in_=table_r[:, ksl, 0:8])
            for g4 in range(KS // 4):
                trp = trppool.tile([8, 4, P], BF16, tag="trp")
                for j in range(4):
                    jj = g4 * 4 + j
                    nc.tensor.transpose(
                        out=trp[:, j, :], in_=rb[:, jj, :],
                        identity=ident_bf[:],
                    )
                trs = trspool.tile([8, 4, P], BF16, tag="trs")
                nc.vector.tensor_copy(out=trs[:], in_=trp[:])
                # software pipeline: matmuls run one group behind the
                # transpose+copy stage so the DVE handoff overlaps PE work
                if pend is not None:
                    emit_mms(*pend)
                pend = (trs, s * (KS // 4) + g4)
        if pend is not None:
            emit_mms(*pend)
        # conf rides in record slots 6:8; one strided gather at the end
        cqr = ppool.tile([P, KT, 2], BF16, tag="cqr")
        nc.scalar.dma_start(out=cqr[:], in_=table_r[:, :, 6:8])
        nc.sync.dma_start(out=cqr_out.ap(), in_=cqr[:])
        trp_psum.__exit__(None, None, None)
        mm_psum.__exit__(None, None, None)

    nc.compile()
    _CACHE["nc"] = nc
    return nc


def make_in_maps(coords, atom_types, W):
    """Shard + pre-pack full inputs into the 8 per-core input maps."""
    coords_flat = np.asarray(coords, dtype=np.float32).reshape(N, 3)
    types_flat = np.asarray(atom_types).reshape(N).astype(np.int32)
    w_np = np.asarray(W, dtype=np.float32)

    w_hi = w_np.astype(BF)
    w_lo = (w_np - w_hi.astype(np.float32)).astype(BF)

    c_hi = coords_flat.astype(BF)
    c_lo = (coords_flat - c_hi.astype(np.float32)).astype(BF)
    conf_g = np.arange(N, dtype=np.int64) // N_ATOMS  # global conformer id
    cq = (conf_g // 64).astype(BF)
    cr = (conf_g % 64).astype(BF)
    # packed record [x_hi y_hi z_hi x_lo y_lo z_lo cq cr] per atom
    rec_all = np.empty((N, 8), dtype=BF)
    rec_all[:, 0:3] = c_hi
    rec_all[:, 3:6] = c_lo
    rec_all[:, 6] = cq
    rec_all[:, 7] = cr

    lmat = np.triu(np.ones((P, P), dtype=np.float32), k=1)
    kk = np.arange(P)
    lblk = (lmat * (kk[:, None] // KS == kk[None, :] // KS)).astype(np.float32)
    sel8 = (np.arange(SS)[:, None] == kk[None, :] // KS).astype(np.float32)
    selT = np.ascontiguousarray(sel8.T)
    ident = np.eye(P, dtype=np.float32)

    in_maps = []
    for s in range(N_CORES):
        sl = slice(s * NS, (s + 1) * NS)
        # [a, k] layouts: atom i = k*128 + a within the core shard
        rec_ak = np.ascontiguousarray(
            rec_all[sl].reshape(KT, P, 8).transpose(1, 0, 2)
        )
        types_ak = np.ascontiguousarray(types_flat[sl].reshape(KT, P).T)
        in_maps.append({
            "rec_in": rec_ak,
            "types_ak": types_ak,
            "w6_in": np.ascontiguousarray(np.concatenate([w_hi, w_hi], axis=0)),
            "wlo_in": np.ascontiguousarray(w_lo),
            "lmat_in": lmat,
            "lblk_in": lblk,
            "sel8_in": sel8,
            "selT_in": selT,
            "ident_in": ident,
        })
    return in_maps


def assemble(results):
    """Stitch the 8 cores x SS sub-shards back into full outputs."""
    counts = np.stack([
        np.rint(res["cnt_out"]).astype(np.int64) for res in results
    ])  # (cores, SS, types)
    at_type_count = counts.sum(axis=(0, 1)).astype(np.int32)

    descs = [res["desc_out"] for res in results]
    confs = [
        (res["cqr_out"][:, :, 0].T.reshape(NS).astype(np.int32) * 64
         + res["cqr_out"][:, :, 1].T.reshape(NS).astype(np.int32))
        for res in results
    ]
    loc_off = np.concatenate(
        [np.zeros((N_CORES, SS, 1), np.int64), np.cumsum(counts, axis=2)],
        axis=2,
    )
    desc_full = np.empty((N, NDESC), dtype=np.float32)
    conf_full = np.empty(N, dtype=np.int32)
    pos = 0
    for t in range(NTYPES):
        for c in range(N_CORES):
            for s in range(SS):
                n = int(counts[c, s, t])
                lo = s * NSS + int(loc_off[c, s, t])
                desc_full[pos:pos + n] = descs[c][lo:lo + n]
                conf_full[pos:pos + n] = confs[c][lo:lo + n]
                pos += n
    assert pos == N
    return desc_full, conf_full, at_type_count


def kernel(coords, atom_types, W):
    nc = build_program()
    in_maps = make_in_maps(coords, atom_types, W)
    res = run_bass_kernel_spmd(nc, in_maps, list(range(N_CORES)))
    return assemble(res.results)


# revision 21
# speedup vs baseline: 1.5943x; 1.0648x over previous
"""Trainium2 Bass kernel for the descriptor-module MoE-routing problem.

Computes, for coords (2048, 64, 3), atom_types (2048, 64), W (3, 384):
    desc_sorted    = tanh(coords.reshape(-1, 3) @ W)[argsort_stable(types)]
    at2conf_sorted = at2conf[argsort_stable(types)]
    at_type_count  = bincount(types, 4)

Sharding: data-parallel over conformers - 8 cores x 256 conformers
(16384 atoms each), and each core further splits its shard into SS=8
independent sub-shards of 16 tiles (2048 atoms). A stable sort by type
decomposes over contiguous index ranges: the global bucket for type t is
the concatenation (in range order) of the per-range buckets, so every
(core, sub-shard) routes independently and the host stitches the 64
bucket lists back together. Sub-sharding lets the Q7 scatter stream of
sub-shard s+1 overlap the compute/write epilogue of sub-shard s.

Numerics: coords and W are split on the host into bf16 hi + lo halves
(x = hi + lo to ~2^-17 relative). The K=3 descriptor matmul runs as
three bf16 matmuls (hi*hi + hi*lo + lo*hi, each product exact in the
f32 PSUM accumulator), giving ~1e-5 error instead of bf16's ~4e-3 -
while running the PE at its full 1 cycle/row bf16 rate (an f32 matmul
is 4x slower, an fp32r one 15x less accurate).

Device algorithm per core:
  1. One batched counting-sort pass over all 8 sub-shards: per-type
     masks, within-tile exclusive ranks (strict-triangular matmul),
     per-sub-shard tile prefix sums (block-diagonal triangular matmul)
     and bucket offsets (selector matmuls). All f32, values < 2048,
     exact. dst[a, k] = sub-shard-local sorted position of every atom.
  2. Routing via the dma_scatter_add Q7 custom instruction (the
     generic vector-indirect DGE path is disabled in this toolchain):
     one 2048-index scatter per sub-shard places 16-byte packed records
     [x_hi y_hi z_hi cq | x_lo y_lo z_lo cr] (8 x bf16) at 256-byte row
     stride into a zeroed table slice. dst is a permutation, so each
     row is written exactly once (CCE-add to zero is exact).
  3. Streaming epilogue per sub-shard: read routed records back,
     bf16 PE-transpose each 128-atom tile (batched 4 tiles per PSUM
     tile + one bulk cast), 3x bf16 matmul with w_hi/w_lo, tanh on
     ScalarE, contiguous write of desc_sorted (25 MB/core stream).
     conf rides along as the exact bf16 pair (q, r) = (conf//64,
     conf%64) and is recombined on the host.
"""

import sys

import numpy as np
import ml_dtypes

try:
    import concourse.bass as bass  # noqa: F401
except ImportError:  # pragma: no cover
    for p in ("/opt/trn_rl_repo", "/root/.axon_site/_ro/trn_rl_repo"):
        if p not in sys.path:
            sys.path.insert(0, p)
    import concourse.bass as bass  # noqa: F401

import concourse.bacc as bacc
import concourse.tile as tile
from concourse import mybir
from concourse.bass_utils import run_bass_kernel_spmd
from concourse.mybir import ActivationFunctionType, AluOpType

N_CORES = 8
N_CONFS = 2048
N_ATOMS = 64
NTYPES = 4
NDESC = 384
N = N_CONFS * N_ATOMS  # 131072
NS = N // N_CORES  # 16384 atoms per core
P = 128
KT = NS // P  # 128 tiles per core
SS = 8  # sub-shards per core (independent sorts)
KS = KT // SS  # 16 tiles per sub-shard
NSS = KS * P  # 2048 atoms per sub-shard
RSTRIDE = 128  # table row stride in bf16 elems (256 B, scatter minimum)
F32 = mybir.dt.float32
BF16 = mybir.dt.bfloat16
I16 = mybir.dt.int16
I32 = mybir.dt.int32
BF = ml_dtypes.bfloat16

_CACHE = {}


def build_program():
    """Build + compile the single-core Bass program (run SPMD on 8 cores)."""
    if "nc" in _CACHE:
        return _CACHE["nc"]

    nc = bacc.Bacc("TRN2", target_bir_lowering=False, debug=False)

    rec_in = nc.dram_tensor("rec_in", [P, KT, 12], BF16, kind="ExternalInput")
    types_in = nc.dram_tensor("types_ak", [P, KT], I32, kind="ExternalInput")
    w9_in = nc.dram_tensor("w9_in", [9, NDESC], BF16, kind="ExternalInput")
    lmat_in = nc.dram_tensor("lmat_in", [P, P], F32, kind="ExternalInput")
    lblk_in = nc.dram_tensor("lblk_in", [P, P], F32, kind="ExternalInput")
    sel8_in = nc.dram_tensor("sel8_in", [SS, P], F32, kind="ExternalInput")
    selT_in = nc.dram_tensor("selT_in", [P, SS], F32, kind="ExternalInput")
    ident_in = nc.dram_tensor("ident_in", [P, P], F32, kind="ExternalInput")

    table = nc.dram_tensor("table", [NS, RSTRIDE], BF16)  # routed records

    desc_out = nc.dram_tensor("desc_out", [NS, NDESC], F32, kind="ExternalOutput")
    cqr_out = nc.dram_tensor("cqr_out", [P, KT, 2], BF16, kind="ExternalOutput")
    cnt_out = nc.dram_tensor("cnt_out", [SS, NTYPES], F32, kind="ExternalOutput")

    table_z = table.ap().rearrange("(s p f) c -> s p (f c)", s=SS, p=P)
    table_r = table.ap().rearrange("(t p) c -> p t c", p=P)
    desc_r = desc_out.ap().rearrange("(t p) d -> p t d", p=P)

    with tile.TileContext(nc) as tc, \
            tc.tile_pool(name="const", bufs=1) as cpool, \
            tc.tile_pool(name="persist", bufs=1) as ppool, \
            tc.tile_pool(name="setup", bufs=2) as spool, \
            tc.tile_pool(name="rb", bufs=3) as rbpool, \
            tc.tile_pool(name="trs", bufs=4) as trspool, \
            tc.tile_pool(name="descp", bufs=3) as descpool:
        setup_psum = tc.tile_pool(name="spsum", bufs=3, space="PSUM")
        sppool = setup_psum.__enter__()
        rank_psum = tc.tile_pool(name="rankps", bufs=2, space="PSUM")
        rkpool = rank_psum.__enter__()

        # ---- constants ----
        w9 = cpool.tile([P, NDESC], BF16, tag="w9")
        nc.scalar.dma_start(out=w9[0:9, :], in_=w9_in.ap())
        lmat = cpool.tile([P, P], F32, tag="lmat")
        nc.sync.dma_start(out=lmat[:], in_=lmat_in.ap())
        lblk = cpool.tile([P, P], F32, tag="lblk")
        nc.scalar.dma_start(out=lblk[:], in_=lblk_in.ap())
        sel8 = cpool.tile([SS, P], F32, tag="sel8")
        nc.scalar.dma_start(out=sel8[:], in_=sel8_in.ap())
        selT = cpool.tile([P, SS], F32, tag="selT")
        nc.scalar.dma_start(out=selT[:], in_=selT_in.ap())
        ident = cpool.tile([P, P], F32, tag="ident")
        nc.sync.dma_start(out=ident[:], in_=ident_in.ap())
        ones = cpool.tile([P, P], F32, tag="ones")
        nc.vector.memset(ones[:], 1.0)
        ident_bf = cpool.tile([P, P], BF16, tag="identbf")
        nc.vector.tensor_copy(out=ident_bf[:], in_=ident[:])
        zeros = cpool.tile([P, RSTRIDE * NSS // P], BF16, tag="zeros")
        nc.vector.memset(zeros[:], 0.0)

        # ---- whole-shard loads (host pre-arranged in [a, k] layout) ----
        rec = ppool.tile([P, KT, 12], BF16, tag="rec")
        nc.sync.dma_start(out=rec[:], in_=rec_in.ap())
        t_i = ppool.tile([P, KT], I32, tag="t_i")
        nc.sync.dma_start(out=t_i[:], in_=types_in.ap())
        t_ak = ppool.tile([P, KT], F32, tag="t_ak")
        nc.vector.tensor_copy(out=t_ak[:], in_=t_i[:])

        # ---- per-type masks over the whole shard ----
        m4 = ppool.tile([P, NTYPES, KT], F32, tag="m4")
        for t in range(NTYPES):
            nc.vector.tensor_scalar(
                out=m4[:, t, :], in0=t_ak[:], scalar1=float(t), scalar2=None,
                op0=AluOpType.is_equal,
            )

        # ---- counts cnt[t, k] -> cntT[k, t] (partition 0 aligned) ----
        cntf_p = sppool.tile([1, NTYPES * KT], F32, tag="ps")
        nc.tensor.matmul(
            out=cntf_p[:], lhsT=ones[:, 0:1], rhs=m4[:], start=True, stop=True
        )
        cntf = spool.tile([1, NTYPES * KT], F32, tag="cntf")
        nc.vector.tensor_copy(out=cntf[:], in_=cntf_p[:])
        cntT = spool.tile([P, NTYPES], F32, tag="cntT")
        for t in range(NTYPES):
            col_p = sppool.tile([P, 1], F32, tag="ps")
            nc.tensor.transpose(
                out=col_p[:], in_=cntf[0:1, t * KT:(t + 1) * KT],
                identity=ident[0:1, 0:1],
            )
            nc.vector.tensor_copy(out=cntT[:, t:t + 1], in_=col_p[:])

        # ---- per-sub-shard totals, bucket offsets, per-tile O ----
        tot_p = sppool.tile([SS, NTYPES], F32, tag="ps")
        nc.tensor.matmul(
            out=tot_p[:], lhsT=selT[:], rhs=cntT[:], start=True, stop=True
        )
        tot = spool.tile([SS, NTYPES], F32, tag="tot")
        nc.vector.tensor_copy(out=tot[:], in_=tot_p[:])
        nc.sync.dma_start(out=cnt_out.ap(), in_=tot[:])
        bofs = spool.tile([SS, NTYPES], F32, tag="bofs")
        nc.vector.memset(bofs[:, 0:1], 0.0)
        for t in range(1, NTYPES):
            nc.vector.tensor_add(
                out=bofs[:, t:t + 1], in0=bofs[:, t - 1:t], in1=tot[:, t - 1:t]
            )
        # O[k, t] = bucket_offset[subshard(k), t] + within-sub-shard prefix
        o_kt_p = sppool.tile([P, NTYPES], F32, tag="ps")
        nc.tensor.matmul(
            out=o_kt_p[:], lhsT=lblk[:], rhs=cntT[:], start=True, stop=False
        )
        nc.tensor.matmul(
            out=o_kt_p[:], lhsT=sel8[:], rhs=bofs[:], start=False, stop=True
        )
        o_kt = spool.tile([P, NTYPES], F32, tag="o_kt")
        nc.vector.tensor_copy(out=o_kt[:], in_=o_kt_p[:])

        # ---- dst[a, k] = sum_t mask_t * (rank_t + O[t, k]) ----
        dst_ak = ppool.tile([P, KT], F32, tag="dst_ak")
        for t in range(NTYPES):
            orow_p = sppool.tile([1, P], F32, tag="ps")
            nc.tensor.transpose(
                out=orow_p[:], in_=o_kt[:, t:t + 1], identity=ident[:]
            )
            o_row = spool.tile([1, P], F32, tag=f"orow{t}")
            nc.vector.tensor_copy(out=o_row[:], in_=orow_p[:])
            rkp = rkpool.tile([P, KT], F32, tag="rank")
            nc.tensor.matmul(
                out=rkp[:], lhsT=lmat[:], rhs=m4[:, t, :], start=True, stop=False
            )
            nc.tensor.matmul(
                out=rkp[:], lhsT=ones[0:1, :], rhs=o_row[0:1, :],
                start=False, stop=True,
            )
            tmp = spool.tile([P, KT], F32, tag="tmp")
            nc.vector.tensor_tensor(
                out=tmp[:], in0=rkp[:], in1=m4[:, t, :], op=AluOpType.mult
            )
            if t == 0:
                nc.vector.tensor_copy(out=dst_ak[:], in_=tmp[:])
            else:
                nc.vector.tensor_add(out=dst_ak[:], in0=dst_ak[:], in1=tmp[:])

        # ---- int16 wrapped index tile for all sub-shards ----
        # per sub-shard scatter record j = k*128 + a; the ucode wraps 16
        # wide: idx16[p, k, q] = dst_ak[q*16 + p%16, k] (values are local
        # to each sub-shard already).
        idx16f = ppool.tile([P, KT, 8], F32, tag="idx16f")
        for q in range(8):
            ext_p = sppool.tile([16, P], F32, tag="ps")
            nc.tensor.matmul(
                out=ext_p[:], lhsT=ident[:, q * 16:(q + 1) * 16], rhs=dst_ak[:],
                start=True, stop=True,
            )
            nc.vector.tensor_copy(out=idx16f[0:16, :, q], in_=ext_p[:])
        idx16 = ppool.tile([P, KT, 8], I16, tag="idx16")
        nc.vector.tensor_copy(out=idx16[0:16, :, :], in_=idx16f[0:16, :, :])
        for lo, hi in ((16, 32), (32, 64), (64, 128)):
            nc.sync.dma_start(
                out=idx16[lo:hi, :, :], in_=idx16[lo - (hi - lo):lo, :, :]
            )

        rank_psum.__exit__(None, None, None)
        setup_psum.__exit__(None, None, None)
        mm_psum = tc.tile_pool(name="mm", bufs=4, space="PSUM")
        mmpool = mm_psum.__enter__()
        trp_psum = tc.tile_pool(name="trp", bufs=3, space="PSUM")
        trppool = trp_psum.__enter__()

        # ---- per-sub-shard: zero, scatter, epilogue ----
        def emit_mms(trs, g):
            dt_ = descpool.tile([P, 4, NDESC], F32, tag="desc")
            for j in range(4):
                mm = mmpool.tile([P, NDESC], F32, tag="mm")
                # hi@w_hi + lo@w_hi + hi@w_lo in a single K=9 matmul
                # (fill cost is N cycles regardless of K)
                nc.tensor.matmul(
                    out=mm[:], lhsT=trs[0:9, j, :], rhs=w9[0:9, :],
                    start=True, stop=True,
                )
                nc.scalar.activation(
                    out=dt_[:, j, :], in_=mm[:],
                    func=ActivationFunctionType.Tanh,
                )
            nc.sync.dma_start(
                out=desc_r[:, g * 4:(g + 1) * 4, :], in_=dt_[:]
            )

        pend = None
        for s in range(SS):
            ksl = slice(s * KS, (s + 1) * KS)
            nc.sync.dma_start(out=table_z[s], in_=zeros[:])
            nc.gpsimd.dma_scatter_add(
                table.ap()[s * NSS:(s + 1) * NSS, 0:12],
                rec[:, ksl, :],
                idx16[:, ksl, :],
                NSS,
                NSS,
                12,
                elem_step=RSTRIDE,
            )

            rb = rbpool.tile([P, KS, 12], BF16, tag="rb")
            nc.scalar.dma_start(out=rb[:], in_=table_r[:, ksl, 0:12])
            for g4 in range(KS // 4):
                trp = trppool.tile([12, 4, P], BF16, tag="trp")
                for j in range(4):
                    jj = g4 * 4 + j
                    nc.tensor.transpose(
                        out=trp[:, j, :], in_=rb[:, jj, :],
                        identity=ident_bf[:],
                    )
                trs = trspool.tile([12, 4, P], BF16, tag="trs")
                nc.vector.tensor_copy(out=trs[:], in_=trp[:])
                # software pipeline: matmuls run one group behind the
                # transpose+copy stage so the DVE handoff overlaps PE work
                if pend is not None:
                    emit_mms(*pend)
                pend = (trs, s * (KS // 4) + g4)
        if pend is not None:
            emit_mms(*pend)
        # conf rides in record slots 6:8; one strided gather at the end
        cqr = ppool.tile([P, KT, 2], BF16, tag="cqr")
        nc.scalar.dma_start(out=cqr[:], in_=table_r[:, :, 9:11])
        nc.sync.dma_start(out=cqr_out.ap(), in_=cqr[:])
        trp_psum.__exit__(None, None, None)
        mm_psum.__exit__(None, None, None)

    nc.compile()
    _CACHE["nc"] = nc
    return nc


def make_in_maps(coords, atom_types, W):
    """Shard + pre-pack full inputs into the 8 per-core input maps."""
    coords_flat = np.asarray(coords, dtype=np.float32).reshape(N, 3)
    types_flat = np.asarray(atom_types).reshape(N).astype(np.int32)
    w_np = np.asarray(W, dtype=np.float32)

    w_hi = w_np.astype(BF)
    w_lo = (w_np - w_hi.astype(np.float32)).astype(BF)

    c_hi = coords_flat.astype(BF)
    c_lo = (coords_flat - c_hi.astype(np.float32)).astype(BF)
    conf_g = np.arange(N, dtype=np.int64) // N_ATOMS  # global conformer id
    cq = (conf_g // 64).astype(BF)
    cr = (conf_g % 64).astype(BF)
    # packed record [x_hi y_hi z_hi | x_lo y_lo z_lo | x_hi y_hi z_hi |
    # cq cr pad]: the hi half is stored twice so the epilogue can compute
    # hi@w_hi + lo@w_hi + hi@w_lo as one K=9 matmul
    rec_all = np.zeros((N, 12), dtype=BF)
    rec_all[:, 0:3] = c_hi
    rec_all[:, 3:6] = c_lo
    rec_all[:, 6:9] = c_hi
    rec_all[:, 9] = cq
    rec_all[:, 10] = cr

    lmat = np.triu(np.ones((P, P), dtype=np.float32), k=1)
    kk = np.arange(P)
    lblk = (lmat * (kk[:, None] // KS == kk[None, :] // KS)).astype(np.float32)
    sel8 = (np.arange(SS)[:, None] == kk[None, :] // KS).astype(np.float32)
    selT = np.ascontiguousarray(sel8.T)
    ident = np.eye(P, dtype=np.float32)

    in_maps = []
    for s in range(N_CORES):
        sl = slice(s * NS, (s + 1) * NS)
        # [a, k] layouts: atom i = k*128 + a within the core shard
        rec_ak = np.ascontiguousarray(
            rec_all[sl].reshape(KT, P, 12).transpose(1, 0, 2)
        )
        types_ak = np.ascontiguousarray(types_flat[sl].reshape(KT, P).T)
        in_maps.append({
            "rec_in": rec_ak,
            "types_ak": types_ak,
            "w9_in": np.ascontiguousarray(
                np.concatenate([w_hi, w_hi, w_lo], axis=0)),
            "lmat_in": lmat,
            "lblk_in": lblk,
            "sel8_in": sel8,
            "selT_in": selT,
            "ident_in": ident,
        })
    return in_maps


def assemble(results):
    """Stitch the 8 cores x SS sub-shards back into full outputs."""
    counts = np.stack([
        np.rint(res["cnt_out"]).astype(np.int64) for res in results
    ])  # (cores, SS, types)
    at_type_count = counts.sum(axis=(0, 1)).astype(np.int32)

    descs = [res["desc_out"] for res in results]
    confs = [
        (res["cqr_out"][:, :, 0].T.reshape(NS).astype(np.int32) * 64
         + res["cqr_out"][:, :, 1].T.reshape(NS).astype(np.int32))
        for res in results
    ]
    loc_off = np.concatenate(
        [np.zeros((N_CORES, SS, 1), np.int64), np.cumsum(counts, axis=2)],
        axis=2,
    )
    desc_full = np.empty((N, NDESC), dtype=np.float32)
    conf_full = np.empty(N, dtype=np.int32)
    pos = 0
    for t in range(NTYPES):
        for c in range(N_CORES):
            for s in range(SS):
                n = int(counts[c, s, t])
                lo = s * NSS + int(loc_off[c, s, t])
                desc_full[pos:pos + n] = descs[c][lo:lo + n]
                conf_full[pos:pos + n] = confs[c][lo:lo + n]
                pos += n
    assert pos == N
    return desc_full, conf_full, at_type_count


def kernel(coords, atom_types, W):
    nc = build_program()
    in_maps = make_in_maps(coords, atom_types, W)
    res = run_bass_kernel_spmd(nc, in_maps, list(range(N_CORES)))
    return assemble(res.results)


# revision 22
# speedup vs baseline: 1.6107x; 1.0102x over previous
"""Trainium2 Bass kernel for the descriptor-module MoE-routing problem.

Computes, for coords (2048, 64, 3), atom_types (2048, 64), W (3, 384):
    desc_sorted    = tanh(coords.reshape(-1, 3) @ W)[argsort_stable(types)]
    at2conf_sorted = at2conf[argsort_stable(types)]
    at_type_count  = bincount(types, 4)

Sharding: data-parallel over conformers - 8 cores x 256 conformers
(16384 atoms each), and each core further splits its shard into SS=8
independent sub-shards of 16 tiles (2048 atoms). A stable sort by type
decomposes over contiguous index ranges: the global bucket for type t is
the concatenation (in range order) of the per-range buckets, so every
(core, sub-shard) routes independently and the host stitches the 64
bucket lists back together. Sub-sharding lets the Q7 scatter stream of
sub-shard s+1 overlap the compute/write epilogue of sub-shard s.

Numerics: coords and W are split on the host into bf16 hi + lo halves
(x = hi + lo to ~2^-17 relative). The K=3 descriptor matmul runs as
three bf16 matmuls (hi*hi + hi*lo + lo*hi, each product exact in the
f32 PSUM accumulator), giving ~1e-5 error instead of bf16's ~4e-3 -
while running the PE at its full 1 cycle/row bf16 rate (an f32 matmul
is 4x slower, an fp32r one 15x less accurate).

Device algorithm per core:
  1. One batched counting-sort pass over all 8 sub-shards: per-type
     masks, within-tile exclusive ranks (strict-triangular matmul),
     per-sub-shard tile prefix sums (block-diagonal triangular matmul)
     and bucket offsets (selector matmuls). All f32, values < 2048,
     exact. dst[a, k] = sub-shard-local sorted position of every atom.
  2. Routing via the dma_scatter_add Q7 custom instruction (the
     generic vector-indirect DGE path is disabled in this toolchain):
     one 2048-index scatter per sub-shard places 16-byte packed records
     [x_hi y_hi z_hi cq | x_lo y_lo z_lo cr] (8 x bf16) at 256-byte row
     stride into a zeroed table slice. dst is a permutation, so each
     row is written exactly once (CCE-add to zero is exact).
  3. Streaming epilogue per sub-shard: read routed records back,
     bf16 PE-transpose each 128-atom tile (batched 4 tiles per PSUM
     tile + one bulk cast), 3x bf16 matmul with w_hi/w_lo, tanh on
     ScalarE, contiguous write of desc_sorted (25 MB/core stream).
     conf rides along as the exact bf16 pair (q, r) = (conf//64,
     conf%64) and is recombined on the host.
"""

import sys

import numpy as np
import ml_dtypes

try:
    import concourse.bass as bass  # noqa: F401
except ImportError:  # pragma: no cover
    for p in ("/opt/trn_rl_repo", "/root/.axon_site/_ro/trn_rl_repo"):
        if p not in sys.path:
            sys.path.insert(0, p)
    import concourse.bass as bass  # noqa: F401

import concourse.bacc as bacc
import concourse.tile as tile
from concourse import mybir
from concourse.bass_utils import run_bass_kernel_spmd
from concourse.mybir import ActivationFunctionType, AluOpType

N_CORES = 8
N_CONFS = 2048
N_ATOMS = 64
NTYPES = 4
NDESC = 384
N = N_CONFS * N_ATOMS  # 131072
NS = N // N_CORES  # 16384 atoms per core
P = 128
KT = NS // P  # 128 tiles per core
SS = 8  # sub-shards per core (independent sorts)
KS = KT // SS  # 16 tiles per sub-shard
NSS = KS * P  # 2048 atoms per sub-shard
RSTRIDE = 128  # table row stride in bf16 elems (256 B, scatter minimum)
F32 = mybir.dt.float32
BF16 = mybir.dt.bfloat16
I16 = mybir.dt.int16
I32 = mybir.dt.int32
BF = ml_dtypes.bfloat16

_CACHE = {}


def build_program():
    """Build + compile the single-core Bass program (run SPMD on 8 cores)."""
    if "nc" in _CACHE:
        return _CACHE["nc"]

    nc = bacc.Bacc("TRN2", target_bir_lowering=False, debug=False)

    rec_in = nc.dram_tensor("rec_in", [P, KT, 12], BF16, kind="ExternalInput")
    types_in = nc.dram_tensor("types_ak", [P, KT], I32, kind="ExternalInput")
    w9_in = nc.dram_tensor("w9_in", [9, NDESC], BF16, kind="ExternalInput")
    lmat_in = nc.dram_tensor("lmat_in", [P, P], F32, kind="ExternalInput")
    lblk_in = nc.dram_tensor("lblk_in", [P, P], F32, kind="ExternalInput")
    sel8_in = nc.dram_tensor("sel8_in", [SS, P], F32, kind="ExternalInput")
    selT_in = nc.dram_tensor("selT_in", [P, SS], F32, kind="ExternalInput")
    ident_in = nc.dram_tensor("ident_in", [P, P], F32, kind="ExternalInput")

    table = nc.dram_tensor("table", [NS, RSTRIDE], BF16)  # routed records

    desc_out = nc.dram_tensor("desc_out", [NS, NDESC], F32, kind="ExternalOutput")
    cqr_out = nc.dram_tensor("cqr_out", [P, KT, 2], BF16, kind="ExternalOutput")
    cnt_out = nc.dram_tensor("cnt_out", [SS, NTYPES], F32, kind="ExternalOutput")

    table_z = table.ap().rearrange("(s p f) c -> s p (f c)", s=SS, p=P)
    table_r = table.ap().rearrange("(t p) c -> p t c", p=P)
    desc_r = desc_out.ap().rearrange("(t p) d -> p t d", p=P)

    with tile.TileContext(nc) as tc, \
            tc.tile_pool(name="const", bufs=1) as cpool, \
            tc.tile_pool(name="persist", bufs=1) as ppool, \
            tc.tile_pool(name="setup", bufs=2) as spool, \
            tc.tile_pool(name="rb", bufs=4) as rbpool, \
            tc.tile_pool(name="trs", bufs=6) as trspool, \
            tc.tile_pool(name="descp", bufs=3) as descpool:
        setup_psum = tc.tile_pool(name="spsum", bufs=3, space="PSUM")
        sppool = setup_psum.__enter__()
        rank_psum = tc.tile_pool(name="rankps", bufs=2, space="PSUM")
        rkpool = rank_psum.__enter__()

        # ---- constants ----
        w9 = cpool.tile([P, NDESC], BF16, tag="w9")
        nc.scalar.dma_start(out=w9[0:9, :], in_=w9_in.ap())
        lmat = cpool.tile([P, P], F32, tag="lmat")
        nc.sync.dma_start(out=lmat[:], in_=lmat_in.ap())
        lblk = cpool.tile([P, P], F32, tag="lblk")
        nc.scalar.dma_start(out=lblk[:], in_=lblk_in.ap())
        sel8 = cpool.tile([SS, P], F32, tag="sel8")
        nc.scalar.dma_start(out=sel8[:], in_=sel8_in.ap())
        selT = cpool.tile([P, SS], F32, tag="selT")
        nc.scalar.dma_start(out=selT[:], in_=selT_in.ap())
        ident = cpool.tile([P, P], F32, tag="ident")
        nc.sync.dma_start(out=ident[:], in_=ident_in.ap())
        ones = cpool.tile([P, P], F32, tag="ones")
        nc.vector.memset(ones[:], 1.0)
        ident_bf = cpool.tile([P, P], BF16, tag="identbf")
        nc.vector.tensor_copy(out=ident_bf[:], in_=ident[:])
        zeros = cpool.tile([P, RSTRIDE * NSS // P], BF16, tag="zeros")
        nc.vector.memset(zeros[:], 0.0)

        # ---- whole-shard loads (host pre-arranged in [a, k] layout) ----
        rec = ppool.tile([P, KT, 12], BF16, tag="rec")
        nc.sync.dma_start(out=rec[:], in_=rec_in.ap())
        t_i = ppool.tile([P, KT], I32, tag="t_i")
        nc.sync.dma_start(out=t_i[:], in_=types_in.ap())
        t_ak = ppool.tile([P, KT], F32, tag="t_ak")
        nc.vector.tensor_copy(out=t_ak[:], in_=t_i[:])

        # ---- per-type masks over the whole shard ----
        m4 = ppool.tile([P, NTYPES, KT], F32, tag="m4")
        for t in range(NTYPES):
            nc.vector.tensor_scalar(
                out=m4[:, t, :], in0=t_ak[:], scalar1=float(t), scalar2=None,
                op0=AluOpType.is_equal,
            )

        # ---- counts cnt[t, k] -> cntT[k, t] (partition 0 aligned) ----
        cntf_p = sppool.tile([1, NTYPES * KT], F32, tag="ps")
        nc.tensor.matmul(
            out=cntf_p[:], lhsT=ones[:, 0:1], rhs=m4[:], start=True, stop=True
        )
        cntf = spool.tile([1, NTYPES * KT], F32, tag="cntf")
        nc.vector.tensor_copy(out=cntf[:], in_=cntf_p[:])
        cntT = spool.tile([P, NTYPES], F32, tag="cntT")
        for t in range(NTYPES):
            col_p = sppool.tile([P, 1], F32, tag="ps")
            nc.tensor.transpose(
                out=col_p[:], in_=cntf[0:1, t * KT:(t + 1) * KT],
                identity=ident[0:1, 0:1],
            )
            nc.vector.tensor_copy(out=cntT[:, t:t + 1], in_=col_p[:])

        # ---- per-sub-shard totals, bucket offsets, per-tile O ----
        tot_p = sppool.tile([SS, NTYPES], F32, tag="ps")
        nc.tensor.matmul(
            out=tot_p[:], lhsT=selT[:], rhs=cntT[:], start=True, stop=True
        )
        tot = spool.tile([SS, NTYPES], F32, tag="tot")
        nc.vector.tensor_copy(out=tot[:], in_=tot_p[:])
        nc.sync.dma_start(out=cnt_out.ap(), in_=tot[:])
        bofs = spool.tile([SS, NTYPES], F32, tag="bofs")
        nc.vector.memset(bofs[:, 0:1], 0.0)
        for t in range(1, NTYPES):
            nc.vector.tensor_add(
                out=bofs[:, t:t + 1], in0=bofs[:, t - 1:t], in1=tot[:, t - 1:t]
            )
        # O[k, t] = bucket_offset[subshard(k), t] + within-sub-shard prefix
        o_kt_p = sppool.tile([P, NTYPES], F32, tag="ps")
        nc.tensor.matmul(
            out=o_kt_p[:], lhsT=lblk[:], rhs=cntT[:], start=True, stop=False
        )
        nc.tensor.matmul(
            out=o_kt_p[:], lhsT=sel8[:], rhs=bofs[:], start=False, stop=True
        )
        o_kt = spool.tile([P, NTYPES], F32, tag="o_kt")
        nc.vector.tensor_copy(out=o_kt[:], in_=o_kt_p[:])

        # ---- dst[a, k] = sum_t mask_t * (rank_t + O[t, k]) ----
        dst_ak = ppool.tile([P, KT], F32, tag="dst_ak")
        for t in range(NTYPES):
            orow_p = sppool.tile([1, P], F32, tag="ps")
            nc.tensor.transpose(
                out=orow_p[:], in_=o_kt[:, t:t + 1], identity=ident[:]
            )
            o_row = spool.tile([1, P], F32, tag=f"orow{t}")
            nc.vector.tensor_copy(out=o_row[:], in_=orow_p[:])
            rkp = rkpool.tile([P, KT], F32, tag="rank")
            nc.tensor.matmul(
                out=rkp[:], lhsT=lmat[:], rhs=m4[:, t, :], start=True, stop=False
            )
            nc.tensor.matmul(
                out=rkp[:], lhsT=ones[0:1, :], rhs=o_row[0:1, :],
                start=False, stop=True,
            )
            tmp = spool.tile([P, KT], F32, tag="tmp")
            nc.vector.tensor_tensor(
                out=tmp[:], in0=rkp[:], in1=m4[:, t, :], op=AluOpType.mult
            )
            if t == 0:
                nc.vector.tensor_copy(out=dst_ak[:], in_=tmp[:])
            else:
                nc.vector.tensor_add(out=dst_ak[:], in0=dst_ak[:], in1=tmp[:])

        # ---- int16 wrapped index tile for all sub-shards ----
        # per sub-shard scatter record j = k*128 + a; the ucode wraps 16
        # wide: idx16[p, k, q] = dst_ak[q*16 + p%16, k] (values are local
        # to each sub-shard already).
        idx16f = ppool.tile([P, KT, 8], F32, tag="idx16f")
        for q in range(8):
            ext_p = sppool.tile([16, P], F32, tag="ps")
            nc.tensor.matmul(
                out=ext_p[:], lhsT=ident[:, q * 16:(q + 1) * 16], rhs=dst_ak[:],
                start=True, stop=True,
            )
            nc.vector.tensor_copy(out=idx16f[0:16, :, q], in_=ext_p[:])
        idx16 = ppool.tile([P, KT, 8], I16, tag="idx16")
        nc.vector.tensor_copy(out=idx16[0:16, :, :], in_=idx16f[0:16, :, :])
        for lo, hi in ((16, 32), (32, 64), (64, 128)):
            nc.sync.dma_start(
                out=idx16[lo:hi, :, :], in_=idx16[lo - (hi - lo):lo, :, :]
            )

        rank_psum.__exit__(None, None, None)
        setup_psum.__exit__(None, None, None)
        mm_psum = tc.tile_pool(name="mm", bufs=4, space="PSUM")
        mmpool = mm_psum.__enter__()
        trp_psum = tc.tile_pool(name="trp", bufs=3, space="PSUM")
        trppool = trp_psum.__enter__()

        # ---- per-sub-shard: zero, scatter, epilogue ----
        def emit_mms(trs, g, dt_, dj):
            for j in range(4):
                mm = mmpool.tile([P, NDESC], F32, tag="mm")
                # hi@w_hi + lo@w_hi + hi@w_lo in a single K=9 matmul
                # (fill cost is N cycles regardless of K)
                nc.tensor.matmul(
                    out=mm[:], lhsT=trs[0:9, j, :], rhs=w9[0:9, :],
                    start=True, stop=True,
                )
                nc.scalar.activation(
                    out=dt_[:, dj * 4 + j, :], in_=mm[:],
                    func=ActivationFunctionType.Tanh,
                )
            if dj == 1:
                nc.sync.dma_start(
                    out=desc_r[:, (g - 1) * 4:(g + 1) * 4, :], in_=dt_[:]
                )

        for s in range(SS):
            ksl = slice(s * KS, (s + 1) * KS)
            nc.sync.dma_start(out=table_z[s], in_=zeros[:])
            nc.gpsimd.dma_scatter_add(
                table.ap()[s * NSS:(s + 1) * NSS, 0:12],
                rec[:, ksl, :],
                idx16[:, ksl, :],
                NSS,
                NSS,
                12,
                elem_step=RSTRIDE,
            )

        pend = None
        for s in range(SS):
            ksl = slice(s * KS, (s + 1) * KS)
            rb = rbpool.tile([P, KS, 12], BF16, tag="rb")
            nc.scalar.dma_start(out=rb[:], in_=table_r[:, ksl, 0:12])
            for g4 in range(KS // 4):
                trp = trppool.tile([12, 4, P], BF16, tag="trp")
                for j in range(4):
                    jj = g4 * 4 + j
                    nc.tensor.transpose(
                        out=trp[:, j, :], in_=rb[:, jj, :],
                        identity=ident_bf[:],
                    )
                trs = trspool.tile([12, 4, P], BF16, tag="trs")
                nc.vector.tensor_copy(out=trs[:], in_=trp[:])
                # software pipeline: matmuls run one group behind the
                # transpose+copy stage so the DVE handoff overlaps PE work
                g = s * (KS // 4) + g4
                if pend is not None:
                    if pend[1] % 2 == 0:
                        dt_ = descpool.tile([P, 8, NDESC], F32, tag="desc")
                    emit_mms(pend[0], pend[1], dt_, pend[1] % 2)
                pend = (trs, g)
        if pend is not None:
            if pend[1] % 2 == 0:
                dt_ = descpool.tile([P, 8, NDESC], F32, tag="desc")
            emit_mms(pend[0], pend[1], dt_, pend[1] % 2)
        # conf rides in record slots 6:8; one strided gather at the end
        cqr = ppool.tile([P, KT, 2], BF16, tag="cqr")
        nc.scalar.dma_start(out=cqr[:], in_=table_r[:, :, 9:11])
        nc.sync.dma_start(out=cqr_out.ap(), in_=cqr[:])
        trp_psum.__exit__(None, None, None)
        mm_psum.__exit__(None, None, None)

    nc.compile()
    _CACHE["nc"] = nc
    return nc


def make_in_maps(coords, atom_types, W):
    """Shard + pre-pack full inputs into the 8 per-core input maps."""
    coords_flat = np.asarray(coords, dtype=np.float32).reshape(N, 3)
    types_flat = np.asarray(atom_types).reshape(N).astype(np.int32)
    w_np = np.asarray(W, dtype=np.float32)

    w_hi = w_np.astype(BF)
    w_lo = (w_np - w_hi.astype(np.float32)).astype(BF)

    c_hi = coords_flat.astype(BF)
    c_lo = (coords_flat - c_hi.astype(np.float32)).astype(BF)
    conf_g = np.arange(N, dtype=np.int64) // N_ATOMS  # global conformer id
    cq = (conf_g // 64).astype(BF)
    cr = (conf_g % 64).astype(BF)
    # packed record [x_hi y_hi z_hi | x_lo y_lo z_lo | x_hi y_hi z_hi |
    # cq cr pad]: the hi half is stored twice so the epilogue can compute
    # hi@w_hi + lo@w_hi + hi@w_lo as one K=9 matmul
    rec_all = np.zeros((N, 12), dtype=BF)
    rec_all[:, 0:3] = c_hi
    rec_all[:, 3:6] = c_lo
    rec_all[:, 6:9] = c_hi
    rec_all[:, 9] = cq
    rec_all[:, 10] = cr

    lmat = np.triu(np.ones((P, P), dtype=np.float32), k=1)
    kk = np.arange(P)
    lblk = (lmat * (kk[:, None] // KS == kk[None, :] // KS)).astype(np.float32)
    sel8 = (np.arange(SS)[:, None] == kk[None, :] // KS).astype(np.float32)
    selT = np.ascontiguousarray(sel8.T)
    ident = np.eye(P, dtype=np.float32)

    in_maps = []
    for s in range(N_CORES):
        sl = slice(s * NS, (s + 1) * NS)
        # [a, k] layouts: atom i = k*128 + a within the core shard
        rec_ak = np.ascontiguousarray(
            rec_all[sl].reshape(KT, P, 12).transpose(1, 0, 2)
        )
        types_ak = np.ascontiguousarray(types_flat[sl].reshape(KT, P).T)
        in_maps.append({
            "rec_in": rec_ak,
            "types_ak": types_ak,
            "w9_in": np.ascontiguousarray(
                np.concatenate([w_hi, w_hi, w_lo], axis=0)),
            "lmat_in": lmat,
            "lblk_in": lblk,
            "sel8_in": sel8,
            "selT_in": selT,
            "ident_in": ident,
        })
    return in_maps


def assemble(results):
    """Stitch the 8 cores x SS sub-shards back into full outputs."""
    counts = np.stack([
        np.rint(res["cnt_out"]).astype(np.int64) for res in results
    ])  # (cores, SS, types)
    at_type_count = counts.sum(axis=(0, 1)).astype(np.int32)

    descs = [res["desc_out"] for res in results]
    confs = [
        (res["cqr_out"][:, :, 0].T.reshape(NS).astype(np.int32) * 64
         + res["cqr_out"][:, :, 1].T.reshape(NS).astype(np.int32))
        for res in results
    ]
    loc_off = np.concatenate(
        [np.zeros((N_CORES, SS, 1), np.int64), np.cumsum(counts, axis=2)],
        axis=2,
    )
    desc_full = np.empty((N, NDESC), dtype=np.float32)
    conf_full = np.empty(N, dtype=np.int32)
    pos = 0
    for t in range(NTYPES):
        for c in range(N_CORES):
            for s in range(SS):
                n = int(counts[c, s, t])
                lo = s * NSS + int(loc_off[c, s, t])
                desc_full[pos:pos + n] = descs[c][lo:lo + n]
                conf_full[pos:pos + n] = confs[c][lo:lo + n]
                pos += n
    assert pos == N
    return desc_full, conf_full, at_type_count


def kernel(coords, atom_types, W):
    nc = build_program()
    in_maps = make_in_maps(coords, atom_types, W)
    res = run_bass_kernel_spmd(nc, in_maps, list(range(N_CORES)))
    return assemble(res.results)
